# revision 10
# baseline (speedup 1.0000x reference)
"""Trainium2 Bass kernel for CausalWanSelfAttention (frame-causal windowed
attention with QK-RMSNorm + RoPE), sharded over 8 NeuronCores.

Sharding: each core owns T = (h*w)/8 tokens of every frame (frame-balanced
interleave).  Each core computes Q/K/V projections + RMSNorm + RoPE for its
own tokens, K/V are exchanged with AllGathers (K whole; V per-frame so the
gathered V is frame-contiguous), attention + O-projection are computed
locally for the core's query tokens.

Device layouts:
  - q/k feature-major [ch, tok] (channels on partitions), with each head's
    128 channels permuted to [re(0..63) | im(0..63)] so RoPE works on
    contiguous partition blocks (permutation is folded into Wq/Wk on host).
  - v token-major [tok, ch] (natural channel order).
  - attention works on frame-contiguous key tiles: per (head, key-frame) a
    single strided DMA assembles K^T [128ch, FRAME] and V [FRAME, 128ch]
    from the gathered buffers; keys are chunked 12x128+tail.
  - scores computed as s^T [keys, q] -> exp on ACT over multi-chunk PSUM
    tiles (2-chunk main segments + shared 7-chunk tail bank).
  - softmax denominator Z accumulated by ones-matmuls into a dedicated
    PSUM bank (rows at partitions 0/32); per-head 1/Z applied by DVE
    during o-PSUM eviction with a partition-broadcast tile.
  - RMSNorm scale r (per token) is folded into the RoPE cos/sin tables;
    per-channel gain g and bias b are folded into the ACT eviction.
"""

import math
import sys
from contextlib import ExitStack

import numpy as np

if "/opt/trn_rl_repo" not in sys.path:
    sys.path.insert(0, "/opt/trn_rl_repo")

import ml_dtypes

BF16 = ml_dtypes.bfloat16
NC = 8  # cores
D = 128  # head dim
EPS = 1e-6


def _chunks(n, width=128):
    return [(g * width, min(n, (g + 1) * width))
            for g in range((n + width - 1) // width)]


# ---------------------------------------------------------------------------
# device program
# ---------------------------------------------------------------------------
_BUILD_CACHE = {}


def build_program(NH, F, T, allowed_kf, cap_waits=True, use_collective=True):
    """Build the SPMD Bass program (identical on all 8 cores).

    NH: number of heads; F: frames; T: tokens per (core, frame);
    allowed_kf[qf] = list of key frames query-frame qf may attend to
    (must make, for each kf, the attending q-set a contiguous suffix of
    frames -- true for causal masks).
    """
    key = (NH, F, T, tuple(tuple(a) for a in allowed_kf), cap_waits,
           use_collective)
    if key in _BUILD_CACHE:
        return _BUILD_CACHE[key]

    import concourse.bass as bass
    import concourse.mybir as mybir
    import concourse.tile as tile
    from concourse.mybir import ActivationFunctionType as AF

    dt = mybir.dt
    DIM = NH * D
    S = F * T              # tokens per core
    FRAME = NC * T         # tokens per frame (= keys per frame)
    H0 = (S + 1) // 2      # token halves for the q/k projections
    SLICE = min(512, DIM)  # out-channel slice for v/o projections
    NSL = DIM // SLICE
    TOKCH = _chunks(S, 128)  # token chunks for v/o projections
    KCH = _chunks(FRAME, 128)  # key chunks within a frame (frame-contiguous)
    BANK = 512             # fp32 elements per PSUM bank

    # for each key frame kf: the first query frame that attends to it
    first_qf = {}
    for kf in range(F):
        qs = [qf for qf in range(F) if kf in allowed_kf[qf]]
        assert qs, f"key frame {kf} unused"
        assert qs == list(range(qs[0], F)), "non-suffix q-set unsupported"
        first_qf[kf] = qs[0]
    q0_min = min(T * first_qf[kf] for kf in range(F))

    # q split: seg-a = [q0(kf), QA), seg-b = [QA, S) shared-tail
    QA = min(BANK, S)
    TAILW = S - QA                        # 73 for S=585
    assert TAILW * 4 <= 2048 or TAILW == 0
    TPB = max(1, (2048 // (TAILW * 4)) if TAILW else 1)  # tail slots per bank

    nc = bass.Bass()

    # ---------------- I/O ----------------
    xT_d = nc.dram_tensor("xT", [DIM, S], dt.bfloat16, kind="ExternalInput")
    w_d = {}
    for nm in ("wqT", "wkT", "wvT", "woT"):
        w_d[nm] = nc.dram_tensor(nm, [DIM, DIM], dt.bfloat16, kind="ExternalInput")
    # packed per-channel affine constants: bq|gq|bq*gq|bk|gk|bk*gk
    bias_d = nc.dram_tensor("bias_pack", [128, 6 * NH], dt.float32,
                            kind="ExternalInput")
    bv_d = nc.dram_tensor("bv_r", [1, DIM], dt.bfloat16, kind="ExternalInput")
    bo_d = nc.dram_tensor("bo_r", [1, DIM], dt.float32, kind="ExternalInput")
    angS_d = nc.dram_tensor("angS", [128, S], dt.float32, kind="ExternalInput")
    angC_d = nc.dram_tensor("angC", [128, S], dt.float32, kind="ExternalInput")
    out_d = nc.dram_tensor("out", [S, DIM], dt.float32, kind="ExternalOutput")

    rg = [list(range(NC))]
    inv_sqrt_d = 1.0 / math.sqrt(D)

    with tile.TileContext(nc) as tc, ExitStack() as ctx:
        dram = ctx.enter_context(tc.tile_pool(name="dram", bufs=1, space="DRAM"))
        k_loc = dram.tile([DIM, S], dt.bfloat16)
        # per-frame V buffers: separate tiles so the per-frame AllGathers
        # never false-share (whole-tile dep tracking) with later writes/reads
        v_loc_f = [dram.tile([T, DIM], dt.bfloat16, name=f"vloc{kf}")
                   for kf in range(F)]
        k_all = dram.tile([NC * DIM, S], dt.bfloat16, addr_space="Shared")
        v_all_f = [dram.tile([FRAME, DIM], dt.bfloat16, addr_space="Shared",
                             name=f"vall{kf}")
                   for kf in range(F)]

        const = ctx.enter_context(tc.tile_pool(name="const", bufs=1))
        resid = ctx.enter_context(tc.tile_pool(name="resid", bufs=1))

        ones_key = const.tile([128, 1], dt.bfloat16)
        nc.vector.memset(ones_key, 1.0)
        ones_f32 = const.tile([128, 1], dt.float32)
        nc.vector.memset(ones_f32, 1.0)
        ones_row = const.tile([1, 128], dt.bfloat16)
        nc.vector.memset(ones_row, 1.0)
        eps_t = const.tile([128, 1], dt.float32)
        nc.vector.memset(eps_t, EPS)

        # constant / bias tiles (one DMA for the packed affine constants)
        bias_sb = const.tile([128, 6 * NH], dt.float32)
        nc.sync.dma_start(out=bias_sb[:], in_=bias_d[:])
        bq_sb = bias_sb[:, 0 * NH:1 * NH]
        gq_sb = bias_sb[:, 1 * NH:2 * NH]
        bqgq_sb = bias_sb[:, 2 * NH:3 * NH]
        bk_sb = bias_sb[:, 3 * NH:4 * NH]
        gk_sb = bias_sb[:, 4 * NH:5 * NH]
        bkgk_sb = bias_sb[:, 5 * NH:6 * NH]
        bv_sb = const.tile([1, DIM], dt.bfloat16)
        nc.sync.dma_start(out=bv_sb[:], in_=bv_d[:])
        bo_bc = const.tile([128, DIM], dt.float32)
        nc.sync.dma_start(
            out=bo_bc[:],
            in_=bass.AP(tensor=bo_d[:].tensor, offset=bo_d[:].offset,
                        ap=[[0, 128]] + bo_d[:].ap[1:]),
        )

        # persistent across phases: rotated q and attention output
        qrot = resid.tile([128, NH, S], dt.bfloat16)
        oT_sb = resid.tile([128, NH, S], dt.bfloat16)

        # prep-phase tensors (freed before attention)
        prep = ExitStack()
        prepp = prep.enter_context(tc.tile_pool(name="prep", bufs=1))

        # x (feature-major), resident through the projections
        xT_sb = prepp.tile([128, NH, S], dt.bfloat16)
        nc.sync.dma_start(out=xT_sb[:], in_=xT_d[:].rearrange("(m p) s -> p m s", p=128))

        # raw RoPE sin/cos (shared q/k)
        angS_sb = prepp.tile([128, S], dt.float32)
        angC_sb = prepp.tile([128, S], dt.float32)
        nc.sync.dma_start(out=angS_sb[:], in_=angS_d[:])
        nc.sync.dma_start(out=angC_sb[:], in_=angC_d[:])
        # angles arrive host-canonicalized to [-pi, pi] (ACT Sin table range)
        sin_raw = prepp.tile([128, S], dt.float32)
        cos_raw = prepp.tile([128, S], dt.float32)
        nc.scalar.activation(sin_raw[:], angS_sb[:], AF.Sin)
        nc.scalar.activation(cos_raw[:], angC_sb[:], AF.Sin)

        qhat = prepp.tile([128, NH, S], dt.bfloat16)
        khat = prepp.tile([128, NH, S], dt.bfloat16)
        krot = prepp.tile([128, NH, S], dt.bfloat16)
        r_q = prepp.tile([1, S], dt.float32)
        r_k = prepp.tile([1, S], dt.float32)

        halves = [(0, H0), (H0, S)] if S > H0 else [(0, S)]

        wpool = prep.enter_context(tc.tile_pool(name="w_qkv", bufs=2))

        # ---------------- Q/K projections + RMS stats ----------------
        def qk_proj(wname, bias_sb, gain_sb, bg_sb, hat, r_sb):
          with ExitStack() as pctx:
            pspool = pctx.enter_context(
                tc.tile_pool(name=f"ps_{wname}", bufs=4, space="PSUM"))
            sspool = pctx.enter_context(
                tc.tile_pool(name=f"ss_{wname}", bufs=2, space="PSUM"))
            evpool = pctx.enter_context(tc.tile_pool(name=f"ev_{wname}", bufs=3))
            w_sb = wpool.tile([128, NH, DIM], dt.bfloat16, tag="w")
            nc.sync.dma_start(
                out=w_sb[:], in_=w_d[wname][:].rearrange("(kc p) n -> p kc n", p=128))
            ss_ps = {}
            for hi, (ha, hb) in enumerate(halves):
                ss_ps[hi] = sspool.tile([1, hb - ha], dt.float32, tag="ss", name=f"ss{hi}")
            for m in range(NH):
                ps = {}
                for hi, (ha, hb) in enumerate(halves):
                    ps[hi] = pspool.tile([128, hb - ha], dt.float32, tag="ps", name=f"ps{hi}")
                for kc in range(NH):
                    for hi, (ha, hb) in enumerate(halves):
                        nc.tensor.matmul(ps[hi][:, :hb - ha],
                                         w_sb[:, kc, m * 128:(m + 1) * 128],
                                         xT_sb[:, kc, ha:hb],
                                         start=(kc == 0), stop=(kc == NH - 1))
                for hi, (ha, hb) in enumerate(halves):
                    hw_ = hb - ha
                    sq = evpool.tile([128, H0], dt.bfloat16, tag="sq")
                    # (q + b)^2
                    nc.scalar.activation(sq[:, :hw_], ps[hi][:, :hw_], AF.Square,
                                         bias=bias_sb[:, m:m + 1])
                    # qhat = (q + b) * g = q*g + b*g
                    nc.scalar.activation(hat[:, m, ha:hb], ps[hi][:, :hw_],
                                         AF.Identity, bias=bg_sb[:, m:m + 1],
                                         scale=gain_sb[:, m:m + 1])
                    nc.tensor.matmul(ss_ps[hi][0:1, :hw_], ones_key[:],
                                     sq[:, :hw_],
                                     start=(m == 0), stop=(m == NH - 1))
            for hi, (ha, hb) in enumerate(halves):
                hw_ = hb - ha
                rt = evpool.tile([1, H0], dt.float32, tag="rt")
                # sqrt(mean(q^2) + eps)
                nc.scalar.activation(rt[0:1, :hw_], ss_ps[hi][0:1, :hw_], AF.Sqrt,
                                     bias=eps_t[0:1, :], scale=1.0 / DIM)
                nc.vector.reciprocal(r_sb[0:1, ha:hb], rt[0:1, :hw_])

        # ---------------- RoPE ----------------
        def rope(hat, rot, r_sb, tag):
          with ExitStack() as pctx:
            rp = pctx.enter_context(tc.tile_pool(name=f"rope_{tag}", bufs=3))
            r_dram = dram.tile([1, S], dt.float32, name=f"rdram_{tag}")
            nc.sync.dma_start(out=r_dram[:], in_=r_sb[0:1, :])
            rb = prepp.tile([128, S], dt.float32, name=f"rb_{tag}")
            nc.sync.dma_start(
                out=rb[:],
                in_=bass.AP(tensor=r_dram.tensor, offset=r_dram[0:1, :].offset,
                            ap=[[0, 128]] + r_dram[0:1, :].ap[1:]))
            ct = prepp.tile([128, S], dt.bfloat16, name=f"cos_{tag}")
            st = prepp.tile([128, S], dt.bfloat16, name=f"sin_{tag}")
            nc.vector.tensor_mul(ct[:], cos_raw[:], rb[:])
            nc.vector.tensor_mul(st[:], sin_raw[:], rb[:])
            for m in range(NH):
                sw = rp.tile([128, S], dt.bfloat16, tag="sw")
                nc.sync.dma_start(out=sw[0:64, :], in_=hat[64:128, m, :])
                nc.sync.dma_start(out=sw[64:128, :], in_=hat[0:64, m, :])
                t1 = rp.tile([128, S], dt.bfloat16, tag="t1")
                t2 = rp.tile([128, S], dt.bfloat16, tag="t2")
                nc.vector.tensor_mul(t1[:], hat[:, m, :], ct[:])
                nc.vector.tensor_mul(t2[:], sw[:], st[:])
                nc.vector.tensor_add(rot[:, m, :], t1[:], t2[:])

        # ---------------- V projection (token-major, frame-ordered) -------
        def v_proj():
          with ExitStack() as pctx:
            pspool = pctx.enter_context(
                tc.tile_pool(name="ps_v", bufs=4, space="PSUM"))
            evpool = pctx.enter_context(tc.tile_pool(name="ev_v", bufs=3))
            w_sb = wpool.tile([128, NH, DIM], dt.bfloat16, tag="w")
            nc.sync.dma_start(
                out=w_sb[:], in_=w_d["wvT"][:].rearrange("(kc p) n -> p kc n", p=128))
            # frame kf's rows are complete once chunks covering [kf*T,(kf+1)*T)
            # are evicted; issue that frame's AllGather right after.
            fr_done = {}
            for kf in range(F):
                last_ti = max(ti for ti, (ta, tb) in enumerate(TOKCH)
                              if ta < (kf + 1) * T)
                fr_done.setdefault(last_ti, []).append(kf)
            for ti, (ta, tb) in enumerate(TOKCH):
                tw = tb - ta
                for sl in range(NSL):
                    ps = pspool.tile([128, SLICE], dt.float32, tag="vps")
                    for kc in range(NH):
                        nc.tensor.matmul(ps[:tw, :], xT_sb[:, kc, ta:tb],
                                         w_sb[:, kc, sl * SLICE:(sl + 1) * SLICE],
                                         start=(kc == 0), stop=False)
                    nc.tensor.matmul(ps[:tw, :], ones_row[0:1, :tw],
                                     bv_sb[0:1, sl * SLICE:(sl + 1) * SLICE],
                                     start=False, stop=True)
                    vt = evpool.tile([128, SLICE], dt.bfloat16, tag="vev")
                    nc.scalar.activation(vt[:tw, :], ps[:tw, :], AF.Copy)
                    # split the eviction at frame boundaries (per-frame tiles)
                    for kf in range(F):
                        ia, ib = max(ta, kf * T), min(tb, (kf + 1) * T)
                        if ia >= ib:
                            continue
                        nc.sync.dma_start(
                            out=v_loc_f[kf][ia - kf * T:ib - kf * T,
                                            sl * SLICE:(sl + 1) * SLICE],
                            in_=vt[ia - ta:ib - ta, :])
                if ti in fr_done and use_collective:
                    for kf in fr_done[ti]:
                        nc.gpsimd.collective_compute(
                            "AllGather", mybir.AluOpType.bypass,
                            ins=[v_loc_f[kf][:]], outs=[v_all_f[kf][:]],
                            replica_groups=rg)

        # ---- phase order: K first (collective early), then V, then Q ----
        qk_proj("wkT", bk_sb, gk_sb, bkgk_sb, khat, r_k)
        rope(khat, krot, r_k, "k")
        for m in range(NH):
            nc.sync.dma_start(out=k_loc[m * 128:(m + 1) * 128, :], in_=krot[:, m, :])
        if use_collective:
            nc.gpsimd.collective_compute(
                "AllGather", mybir.AluOpType.bypass, ins=[k_loc[:]],
                outs=[k_all[:]], replica_groups=rg)
        v_proj()
        qk_proj("wqT", bq_sb, gq_sb, bqgq_sb, qhat, r_q)
        rope(qhat, qrot, r_q, "q")
        prep.close()  # free x / hats / krot / angles before attention

        # ---------------- attention ----------------
        # Per (head, key-frame): one K tile [128, FRAME] (strided gather over
        # cores) and one V tile [128, nch, 128] token-major.  Keys chunked
        # 12x128+tail.  Scores s^T [keys, q] into 2-chunk PSUM tiles
        # (seg-a, q in [q0, QA)) plus a shared tail bank (seg-b, q in
        # [QA, S), TPB chunk-slots per bank).  exp on ACT per PSUM tile.
        # o accumulated per head in 2 banks; z by ones-matmuls into 1 bank.
        actx = ExitStack()
        att_k = actx.enter_context(tc.tile_pool(name="att_k", bufs=3))
        att_v = actx.enter_context(tc.tile_pool(name="att_v", bufs=3))
        att_s = actx.enter_context(tc.tile_pool(name="att_s", bufs=2, space="PSUM"))
        att_st = actx.enter_context(tc.tile_pool(name="att_st", bufs=1, space="PSUM"))
        att_o = actx.enter_context(tc.tile_pool(name="att_o", bufs=1, space="PSUM"))
        att_z = actx.enter_context(tc.tile_pool(name="att_z", bufs=1, space="PSUM"))
        att_p = actx.enter_context(tc.tile_pool(name="att_p", bufs=2))
        att_m = actx.enter_context(tc.tile_pool(name="att_m", bufs=2))

        NKC = len(KCH)
        assert q0_min == 0, "oT zero-fill for q < q0_min not implemented"
        for h in range(NH):
            o_a = att_o.tile([128, QA - q0_min], dt.float32, tag="oa", name="oa")
            o_b = (att_o.tile([128, TAILW], dt.float32, tag="ob", name="ob")
                   if TAILW else None)
            z_t = att_z.tile([128, BANK], dt.float32, tag="z", name="z")
            # z rows: row0 at partition 0 covers q [q0_min, QA); row1 at
            # partition 32 covers q [QA, S)

            for kf in range(F):
                q0 = T * first_qf[kf]
                kr_t = att_k.tile([128, NC * T], dt.bfloat16, tag="kr")
                nc.sync.dma_start(
                    out=kr_t[:].rearrange("p (c t) -> p c t", c=NC),
                    in_=k_all[:, kf * T:(kf + 1) * T]
                    .rearrange("(c m p) t -> p c m t", c=NC, p=128)[:, :, h, :])
                v_t = att_v.tile([128, NKC, 128], dt.bfloat16, tag="vt")
                nfull = FRAME // 128
                v_view = v_all_f[kf][:, h * 128:(h + 1) * 128]
                nc.sync.dma_start(
                    out=v_t[:, :nfull, :],
                    in_=v_view[:nfull * 128, :].rearrange("(j p) n -> p j n", p=128))
                if FRAME % 128:
                    nc.sync.dma_start(
                        out=v_t[:FRAME % 128, nfull:nfull + 1, :],
                        in_=v_view[nfull * 128:, :].rearrange(
                            "(j p) n -> p j n", p=FRAME % 128))

                # process chunks in pairs for seg-a; tail-bank groups of TPB
                ci = 0
                while ci < NKC:
                    pair = [c for c in (ci, ci + 1) if c < NKC]
                    s_t = att_s.tile([128, 2, BANK], dt.float32, tag="s")
                    p_t = att_p.tile([128, 2, S], dt.bfloat16, tag="p")
                    for i, c in enumerate(pair):
                        ka, kb = KCH[c]
                        kw = kb - ka
                        nc.tensor.matmul(s_t[:kw, i, :QA - q0],
                                         kr_t[:, ka:kb],
                                         qrot[:, h, q0:QA],
                                         start=True, stop=True)
                    kw_hi = max(KCH[c][1] - KCH[c][0] for c in pair)
                    nc.scalar.activation(
                        p_t[:kw_hi, :len(pair), q0:QA],
                        s_t[:kw_hi, :len(pair), :QA - q0],
                        AF.Exp, scale=inv_sqrt_d)
                    for i, c in enumerate(pair):
                        ka, kb = KCH[c]
                        kw = kb - ka
                        first = (kf == 0 and c == 0)
                        last = (kf == F - 1 and c == NKC - 1)
                        # o / z accumulation (seg-a)
                        nc.tensor.matmul(o_a[:, q0 - q0_min:QA - q0_min],
                                         v_t[:kw, c, :],
                                         p_t[:kw, i, q0:QA],
                                         start=first, stop=last)
                        nc.tensor.matmul(z_t[0:1, q0 - q0_min:QA - q0_min],
                                         ones_key[:kw, :],
                                         p_t[:kw, i, q0:QA],
                                         start=first, stop=last)
                    ci += 2

                # seg-b (tail q columns) in TPB-chunk groups
                if TAILW:
                    ci = 0
                    while ci < NKC:
                        grp = list(range(ci, min(ci + TPB, NKC)))
                        st_t = att_st.tile([128, TPB, TAILW], dt.float32, tag="st")
                        pt_t = att_p.tile([128, TPB, TAILW], dt.bfloat16, tag="pt")
                        for i, c in enumerate(grp):
                            ka, kb = KCH[c]
                            kw = kb - ka
                            nc.tensor.matmul(st_t[:kw, i, :],
                                             kr_t[:, ka:kb],
                                             qrot[:, h, QA:S],
                                             start=True, stop=True)
                        kw_hi = max(KCH[c][1] - KCH[c][0] for c in grp)
                        nc.scalar.activation(
                            pt_t[:kw_hi, :len(grp), :],
                            st_t[:kw_hi, :len(grp), :],
                            AF.Exp, scale=inv_sqrt_d)
                        for i, c in enumerate(grp):
                            ka, kb = KCH[c]
                            kw = kb - ka
                            first = (kf == 0 and c == 0)
                            last = (kf == F - 1 and c == NKC - 1)
                            nc.tensor.matmul(o_b[:, :], v_t[:kw, c, :],
                                             pt_t[:kw, i, :],
                                             start=first, stop=last)
                            nc.tensor.matmul(z_t[32:33, :TAILW],
                                             ones_key[:kw, :],
                                             pt_t[:kw, i, :],
                                             start=first, stop=last)
                        ci += TPB

            # 1/Z and eviction for head h
            z_sb = att_m.tile([1, S], dt.float32, tag="zsb", name="zsb")
            nc.scalar.activation(z_sb[0:1, q0_min:QA],
                                 z_t[0:1, :QA - q0_min], AF.Copy)
            if TAILW:
                nc.vector.tensor_copy(z_sb[0:1, QA:S], z_t[32:33, :TAILW])
            nc.vector.reciprocal(z_sb[0:1, :], z_sb[0:1, :])
            z_dram = dram.tile([1, S], dt.float32, tag="zdram", bufs=2,
                               name="zdram")
            nc.sync.dma_start(out=z_dram[0:1, :], in_=z_sb[0:1, :])
            izb = att_m.tile([128, S], dt.float32, tag="izb", name="izb")
            nc.sync.dma_start(
                out=izb[:],
                in_=bass.AP(tensor=z_dram.tensor, offset=z_dram[0:1, :].offset,
                            ap=[[0, 128]] + z_dram[0:1, :].ap[1:]))
            nc.vector.tensor_mul(oT_sb[:, h, q0_min:QA],
                                 o_a[:, :], izb[:, q0_min:QA])
            if TAILW:
                nc.vector.tensor_mul(oT_sb[:, h, QA:S], o_b[:, :],
                                     izb[:, QA:S])

        actx.close()  # release attention PSUM banks before the O-projection

        # ---------------- O projection ----------------
        wopool = ctx.enter_context(tc.tile_pool(name="w_o", bufs=3))
        pspool = ctx.enter_context(
            tc.tile_pool(name="ps_o", bufs=len(TOKCH) + 1, space="PSUM"))
        evpool = ctx.enter_context(tc.tile_pool(name="ev_o", bufs=3))
        for sl in range(NSL):
            ps = {}
            for ti in range(len(TOKCH)):
                ps[ti] = pspool.tile([128, SLICE], dt.float32, tag="ops", name=f"ops{ti}")
            for m in range(NH):
                wt = wopool.tile([128, SLICE], dt.bfloat16, tag="wo")
                nc.sync.dma_start(
                    out=wt[:],
                    in_=w_d["woT"][m * 128:(m + 1) * 128,
                                   sl * SLICE:(sl + 1) * SLICE])
                for ti, (ta, tb) in enumerate(TOKCH):
                    nc.tensor.matmul(ps[ti][:tb - ta, :], oT_sb[:, m, ta:tb],
                                     wt[:], start=(m == 0), stop=(m == NH - 1))
            for ti, (ta, tb) in enumerate(TOKCH):
                tw = tb - ta
                ot = evpool.tile([128, SLICE], dt.float32, tag="oev")
                nc.vector.tensor_add(ot[:tw, :], ps[ti][:tw, :],
                                     bo_bc[:tw, sl * SLICE:(sl + 1) * SLICE])
                nc.sync.dma_start(
                    out=out_d[ta:tb, sl * SLICE:(sl + 1) * SLICE],
                    in_=ot[:tw, :])

    if cap_waits:
        _cap_sync_waits(nc, mybir)
    _BUILD_CACHE[key] = nc
    return nc


def _cap_sync_waits(nc, mybir, cap=1):
    """Walrus engine-instruction structs only have a limited number of sync
    wait slots.  Hoist excess waits onto InstNoOp carriers placed immediately
    before the instruction on the same engine stream."""
    exempt = (mybir.InstNoOp, mybir.InstEventSemaphore,
              mybir.InstAllEngineBarrier)
    for f in nc.m.functions:
        for bb in f.blocks:
            out = []
            changed = False
            for inst in bb.instructions:
                si = inst.sync_info
                if (si is None or len(si.on_wait) <= cap
                        or isinstance(inst, exempt)):
                    out.append(inst)
                    continue
                waits = list(si.on_wait)
                keep, excess = waits[:cap], waits[cap:]
                while excess:
                    batch, excess = excess[:cap], excess[cap:]
                    out.append(mybir.InstNoOp(
                        name=f"{inst.name}-w{len(out)}",
                        engine=inst.engine,
                        bass_nofuse=True,
                        sync_info=mybir.SyncInfo(on_wait=batch, on_update=[]),
                    ))
                inst.sync_info = mybir.SyncInfo(on_wait=keep,
                                                on_update=list(si.on_update))
                out.append(inst)
                changed = True
            if changed:
                bb.instructions = out


# ---------------------------------------------------------------------------
# host side
# ---------------------------------------------------------------------------
def _perm(NH):
    p = np.empty(NH * D, np.int64)
    for hh in range(NH):
        base = hh * D
        for j in range(D // 2):
            p[base + j] = base + 2 * j
            p[base + D // 2 + j] = base + 2 * j + 1
    return p


def _host_inputs(x, freqs, Wq, bq, Wk, bk, Wv, bv, Wo, bo, gq, gk,
                 f, h, w, num_heads, local_attn_size, sink_size, start_frame):
    NH = num_heads
    DIM = NH * D
    FRAME = h * w
    assert FRAME % NC == 0
    T = FRAME // NC
    S = f * T
    perm = _perm(NH)

    def bf(a):
        return np.ascontiguousarray(a, dtype=np.float32).astype(BF16)

    wqT = bf(Wq[perm].T)
    wkT = bf(Wk[perm].T)
    wvT = bf(Wv.T)
    woT = bf(Wo.T)
    def chunkmajor(a):
        return np.asarray(a, np.float32)[perm].reshape(NH, D).T
    bias_pack = np.ascontiguousarray(np.concatenate(
        [chunkmajor(bq), chunkmajor(gq), chunkmajor(bq) * chunkmajor(gq),
         chunkmajor(bk), chunkmajor(gk), chunkmajor(bk) * chunkmajor(gk)],
        axis=1), np.float32)
    bv_r = bf(bv.reshape(1, DIM))
    bo_r = np.ascontiguousarray(bo.reshape(1, DIM), np.float32)

    c = D // 2
    c1 = c // 3
    c0 = c - 2 * c1
    freqs = np.asarray(freqs, np.float32)

    in_maps = []
    tok_idx = []
    for core in range(NC):
        idx = np.concatenate(
            [fr * FRAME + T * core + np.arange(T) for fr in range(f)])
        tok_idx.append(idx)
        xT = bf(np.asarray(x[0], np.float32)[idx].T)
        fr = idx // FRAME
        rem = idx % FRAME
        hh_i = rem // w
        ww_i = rem % w
        ang = np.empty((c, S), np.float32)
        ang[:c0, :] = freqs[start_frame + fr][:, :c0].T
        ang[c0:c0 + c1, :] = freqs[hh_i][:, c0:c0 + c1].T
        ang[c0 + c1:, :] = freqs[ww_i][:, c0 + c1:c].T
        def wrap(a):
            a = np.asarray(a, np.float64)
            return (a - 2 * np.pi * np.round(a / (2 * np.pi))).astype(np.float32)
        # top half encodes -sin via the (ang + pi) phase shift
        angS = np.ascontiguousarray(
            np.concatenate([wrap(ang + np.pi), wrap(ang)], 0), np.float32)
        angC = np.ascontiguousarray(
            np.concatenate([wrap(ang + np.pi / 2), wrap(ang + np.pi / 2)], 0),
            np.float32)
        in_maps.append({
            "xT": xT, "wqT": wqT, "wkT": wkT, "wvT": wvT, "woT": woT,
            "bias_pack": bias_pack,
            "bv_r": bv_r, "bo_r": bo_r, "angS": angS, "angC": angC,
        })
    return in_maps, tok_idx, T, S


def _allowed(f, local_attn_size, sink_size):
    return [
        [kf for kf in range(f)
         if kf <= qf and (qf - kf < local_attn_size or kf < sink_size)]
        for qf in range(f)
    ]


def kernel(x, freqs, Wq, bq, Wk, bk, Wv, bv, Wo, bo, gq, gk,
           f, h, w, num_heads, local_attn_size, sink_size, start_frame,
           _trace=False):
    from concourse.bass_utils import run_bass_kernel_spmd

    f = int(f); h = int(h); w = int(w)
    num_heads = int(num_heads)
    local_attn_size = int(local_attn_size)
    sink_size = int(sink_size)
    start_frame = int(start_frame)

    x = np.asarray(x)
    B, L, DIM = x.shape
    assert B == 1 and DIM == num_heads * D

    allowed = _allowed(f, local_attn_size, sink_size)
    in_maps, tok_idx, T, S = _host_inputs(
        x, freqs, Wq, bq, Wk, bk, Wv, bv, Wo, bo, gq, gk,
        f, h, w, num_heads, local_attn_size, sink_size, start_frame)
    nc = build_program(num_heads, f, T, allowed)
    res = run_bass_kernel_spmd(nc, in_maps, core_ids=list(range(NC)),
                               trace=_trace)
    out = np.empty((1, L, DIM), np.float32)
    for core in range(NC):
        out[0, tok_idx[core]] = res.results[core]["out"]
    if _trace:
        kernel._last_results = res
    return out


# revision 16
# speedup vs baseline: 1.2756x; 1.2756x over previous
"""Trainium2 Bass kernel for CausalWanSelfAttention (frame-causal windowed
attention with QK-RMSNorm + RoPE), sharded over 8 NeuronCores.

Sharding: each core owns T = (h*w)/8 tokens of every frame (frame-balanced
interleave).  Each core computes Q/K/V projections + RMSNorm + RoPE for its
own tokens, K/V are exchanged with AllGathers (K whole; V per-frame so the
gathered V is frame-contiguous), attention + O-projection are computed
locally for the core's query tokens.

Device layouts:
  - q/k feature-major [ch, tok] (channels on partitions), with each head's
    128 channels permuted to [re(0..63) | im(0..63)] so RoPE works on
    contiguous partition blocks (permutation is folded into Wq/Wk on host).
  - v token-major [tok, ch] (natural channel order).
  - attention works on frame-contiguous key tiles: per (head, key-frame) a
    single strided DMA assembles K^T [128ch, FRAME] and V [FRAME, 128ch]
    from the gathered buffers; keys are chunked 12x128+tail.
  - scores computed as s^T [keys, q] -> exp on ACT over multi-chunk PSUM
    tiles (2-chunk main segments + shared 7-chunk tail bank).
  - softmax denominator Z accumulated by ones-matmuls into a dedicated
    PSUM bank (rows at partitions 0/32); per-head 1/Z applied by DVE
    during o-PSUM eviction with a partition-broadcast tile.
  - RMSNorm scale r (per token) is folded into the RoPE cos/sin tables;
    per-channel gain g and bias b are folded into the ACT eviction.
"""

import math
import sys
from contextlib import ExitStack

import numpy as np

if "/opt/trn_rl_repo" not in sys.path:
    sys.path.insert(0, "/opt/trn_rl_repo")

import ml_dtypes

BF16 = ml_dtypes.bfloat16
NC = 8  # cores
D = 128  # head dim
EPS = 1e-6


def _chunks(n, width=128):
    return [(g * width, min(n, (g + 1) * width))
            for g in range((n + width - 1) // width)]


# ---------------------------------------------------------------------------
# device program
# ---------------------------------------------------------------------------
_BUILD_CACHE = {}


def build_program(NH, F, T, allowed_kf, cap_waits=True, use_collective=True):
    """Build the SPMD Bass program (identical on all 8 cores).

    NH: number of heads; F: frames; T: tokens per (core, frame);
    allowed_kf[qf] = list of key frames query-frame qf may attend to
    (must make, for each kf, the attending q-set a contiguous suffix of
    frames -- true for causal masks).
    """
    key = (NH, F, T, tuple(tuple(a) for a in allowed_kf), cap_waits,
           use_collective)
    if key in _BUILD_CACHE:
        return _BUILD_CACHE[key]

    import concourse.bass as bass
    import concourse.mybir as mybir
    import concourse.tile as tile
    from concourse.mybir import ActivationFunctionType as AF

    dt = mybir.dt
    DIM = NH * D
    S = F * T              # tokens per core
    FRAME = NC * T         # tokens per frame (= keys per frame)
    H0 = (S + 1) // 2      # token halves for the q/k projections
    SLICE = min(512, DIM)  # out-channel slice for v/o projections
    NSL = DIM // SLICE
    TOKCH = _chunks(S, 128)  # token chunks for v/o projections
    KCH = _chunks(FRAME, 128)  # key chunks within a frame (frame-contiguous)
    BANK = 512             # fp32 elements per PSUM bank

    # for each key frame kf: the first query frame that attends to it
    first_qf = {}
    for kf in range(F):
        qs = [qf for qf in range(F) if kf in allowed_kf[qf]]
        assert qs, f"key frame {kf} unused"
        assert qs == list(range(qs[0], F)), "non-suffix q-set unsupported"
        first_qf[kf] = qs[0]
    q0_min = min(T * first_qf[kf] for kf in range(F))

    # q split: seg-a = [q0(kf), QA), seg-b = [QA, S) shared-tail
    QA = min(BANK, S)
    TAILW = S - QA                        # 73 for S=585
    assert TAILW * 4 <= 2048 or TAILW == 0
    TPB = max(1, (2048 // (TAILW * 4)) if TAILW else 1)  # tail slots per bank

    nc = bass.Bass()

    # ---------------- I/O ----------------
    xT_d = nc.dram_tensor("xT", [DIM, S], dt.bfloat16, kind="ExternalInput")
    w_d = {}
    for nm in ("wqT", "wkT", "wvT", "woT"):
        w_d[nm] = nc.dram_tensor(nm, [DIM, DIM], dt.bfloat16, kind="ExternalInput")
    # packed per-channel affine constants: bq|gq|bq*gq|bk|gk|bk*gk
    bias_d = nc.dram_tensor("bias_pack", [128, 6 * NH], dt.float32,
                            kind="ExternalInput")
    bv_d = nc.dram_tensor("bv_r", [1, DIM], dt.bfloat16, kind="ExternalInput")
    bo_d = nc.dram_tensor("bo_r", [1, DIM], dt.float32, kind="ExternalInput")
    angS_d = nc.dram_tensor("angS", [128, S], dt.float32, kind="ExternalInput")
    angC_d = nc.dram_tensor("angC", [128, S], dt.float32, kind="ExternalInput")
    out_d = nc.dram_tensor("out", [S, DIM], dt.float32, kind="ExternalOutput")

    rg = [list(range(NC))]
    inv_sqrt_d = 1.0 / math.sqrt(D)

    with tile.TileContext(nc) as tc, ExitStack() as ctx:
        dram = ctx.enter_context(tc.tile_pool(name="dram", bufs=1, space="DRAM"))
        k_loc = dram.tile([DIM, S], dt.bfloat16)
        # per-frame V buffers: separate tiles so the per-frame AllGathers
        # never false-share (whole-tile dep tracking) with later writes/reads
        v_loc_f = [dram.tile([T, DIM], dt.bfloat16, name=f"vloc{kf}")
                   for kf in range(F)]
        k_all = dram.tile([NC * DIM, S], dt.bfloat16, addr_space="Shared")
        v_all_f = [dram.tile([FRAME, DIM], dt.bfloat16, addr_space="Shared",
                             name=f"vall{kf}")
                   for kf in range(F)]

        const = ctx.enter_context(tc.tile_pool(name="const", bufs=1))
        resid = ctx.enter_context(tc.tile_pool(name="resid", bufs=1))

        ones_key = const.tile([128, 1], dt.bfloat16)
        nc.vector.memset(ones_key, 1.0)
        ones_f32 = const.tile([128, 1], dt.float32)
        nc.vector.memset(ones_f32, 1.0)
        ones_row = const.tile([1, 128], dt.bfloat16)
        nc.vector.memset(ones_row, 1.0)
        eps_t = const.tile([128, 1], dt.float32)
        nc.vector.memset(eps_t, EPS)

        # constant / bias tiles (one DMA for the packed affine constants)
        bias_sb = const.tile([128, 6 * NH], dt.float32)
        nc.sync.dma_start(out=bias_sb[:], in_=bias_d[:])
        bq_sb = bias_sb[:, 0 * NH:1 * NH]
        gq_sb = bias_sb[:, 1 * NH:2 * NH]
        bqgq_sb = bias_sb[:, 2 * NH:3 * NH]
        bk_sb = bias_sb[:, 3 * NH:4 * NH]
        gk_sb = bias_sb[:, 4 * NH:5 * NH]
        bkgk_sb = bias_sb[:, 5 * NH:6 * NH]
        bv_sb = const.tile([1, DIM], dt.bfloat16)
        nc.sync.dma_start(out=bv_sb[:], in_=bv_d[:])
        bo_bc = const.tile([128, DIM], dt.float32)
        nc.sync.dma_start(
            out=bo_bc[:],
            in_=bass.AP(tensor=bo_d[:].tensor, offset=bo_d[:].offset,
                        ap=[[0, 128]] + bo_d[:].ap[1:]),
        )

        # persistent across phases: rotated q and attention output
        qrot = resid.tile([128, NH, S], dt.bfloat16)
        oT_sb = resid.tile([128, NH, S], dt.bfloat16)

        # prep-phase tensors (freed before attention)
        prep = ExitStack()
        prepp = prep.enter_context(tc.tile_pool(name="prep", bufs=1))

        # x (feature-major), resident through the projections
        xT_sb = prepp.tile([128, NH, S], dt.bfloat16)
        nc.sync.dma_start(out=xT_sb[:], in_=xT_d[:].rearrange("(m p) s -> p m s", p=128))

        # raw RoPE sin/cos (shared q/k)
        angS_sb = prepp.tile([128, S], dt.float32)
        angC_sb = prepp.tile([128, S], dt.float32)
        nc.sync.dma_start(out=angS_sb[:], in_=angS_d[:])
        nc.sync.dma_start(out=angC_sb[:], in_=angC_d[:])
        # angles arrive host-canonicalized to [-pi, pi] (ACT Sin table range)
        sin_raw = prepp.tile([128, S], dt.float32)
        cos_raw = prepp.tile([128, S], dt.float32)
        nc.scalar.activation(sin_raw[:], angS_sb[:], AF.Sin)
        nc.scalar.activation(cos_raw[:], angC_sb[:], AF.Sin)

        qhat = prepp.tile([128, NH, S], dt.bfloat16)
        khat = prepp.tile([128, NH, S], dt.bfloat16)
        krot = prepp.tile([128, NH, S], dt.bfloat16)
        r_q = prepp.tile([1, S], dt.float32)
        r_k = prepp.tile([1, S], dt.float32)

        halves = [(0, H0), (H0, S)] if S > H0 else [(0, S)]

        wpool = prep.enter_context(tc.tile_pool(name="w_qkv", bufs=2))

        # ---------------- Q/K projections + RMS stats ----------------
        def qk_proj(wname, bias_sb, gain_sb, bg_sb, hat, r_sb):
          with ExitStack() as pctx:
            pspool = pctx.enter_context(
                tc.tile_pool(name=f"ps_{wname}", bufs=4, space="PSUM"))
            sspool = pctx.enter_context(
                tc.tile_pool(name=f"ss_{wname}", bufs=2, space="PSUM"))
            evpool = pctx.enter_context(tc.tile_pool(name=f"ev_{wname}", bufs=3))
            w_sb = wpool.tile([128, NH, DIM], dt.bfloat16, tag="w")
            nc.sync.dma_start(
                out=w_sb[:], in_=w_d[wname][:].rearrange("(kc p) n -> p kc n", p=128))
            ss_ps = {}
            for hi, (ha, hb) in enumerate(halves):
                ss_ps[hi] = sspool.tile([1, hb - ha], dt.float32, tag="ss", name=f"ss{hi}")
            for m in range(NH):
                ps = {}
                for hi, (ha, hb) in enumerate(halves):
                    ps[hi] = pspool.tile([128, hb - ha], dt.float32, tag="ps", name=f"ps{hi}")
                for kc in range(NH):
                    for hi, (ha, hb) in enumerate(halves):
                        nc.tensor.matmul(ps[hi][:, :hb - ha],
                                         w_sb[:, kc, m * 128:(m + 1) * 128],
                                         xT_sb[:, kc, ha:hb],
                                         start=(kc == 0), stop=(kc == NH - 1))
                for hi, (ha, hb) in enumerate(halves):
                    hw_ = hb - ha
                    sq = evpool.tile([128, H0], dt.bfloat16, tag="sq")
                    # (q + b)^2
                    nc.scalar.activation(sq[:, :hw_], ps[hi][:, :hw_], AF.Square,
                                         bias=bias_sb[:, m:m + 1])
                    # qhat = (q + b) * g = q*g + b*g
                    nc.scalar.activation(hat[:, m, ha:hb], ps[hi][:, :hw_],
                                         AF.Identity, bias=bg_sb[:, m:m + 1],
                                         scale=gain_sb[:, m:m + 1])
                    nc.tensor.matmul(ss_ps[hi][0:1, :hw_], ones_key[:],
                                     sq[:, :hw_],
                                     start=(m == 0), stop=(m == NH - 1))
            for hi, (ha, hb) in enumerate(halves):
                hw_ = hb - ha
                rt = evpool.tile([1, H0], dt.float32, tag="rt")
                # sqrt(mean(q^2) + eps)
                nc.scalar.activation(rt[0:1, :hw_], ss_ps[hi][0:1, :hw_], AF.Sqrt,
                                     bias=eps_t[0:1, :], scale=1.0 / DIM)
                nc.vector.reciprocal(r_sb[0:1, ha:hb], rt[0:1, :hw_])

        # ---------------- RoPE ----------------
        def rope(hat, rot, r_sb, tag):
          with ExitStack() as pctx:
            rp = pctx.enter_context(tc.tile_pool(name=f"rope_{tag}", bufs=3))
            r_dram = dram.tile([1, S], dt.float32, name=f"rdram_{tag}")
            nc.sync.dma_start(out=r_dram[:], in_=r_sb[0:1, :])
            rb = prepp.tile([128, S], dt.float32, name=f"rb_{tag}")
            nc.sync.dma_start(
                out=rb[:],
                in_=bass.AP(tensor=r_dram.tensor, offset=r_dram[0:1, :].offset,
                            ap=[[0, 128]] + r_dram[0:1, :].ap[1:]))
            ct = prepp.tile([128, S], dt.bfloat16, name=f"cos_{tag}")
            st = prepp.tile([128, S], dt.bfloat16, name=f"sin_{tag}")
            nc.vector.tensor_mul(ct[:], cos_raw[:], rb[:])
            nc.vector.tensor_mul(st[:], sin_raw[:], rb[:])
            for m in range(NH):
                sw = rp.tile([128, S], dt.bfloat16, tag="sw")
                nc.sync.dma_start(out=sw[0:64, :], in_=hat[64:128, m, :])
                nc.sync.dma_start(out=sw[64:128, :], in_=hat[0:64, m, :])
                t1 = rp.tile([128, S], dt.bfloat16, tag="t1")
                t2 = rp.tile([128, S], dt.bfloat16, tag="t2")
                nc.vector.tensor_mul(t1[:], hat[:, m, :], ct[:])
                nc.vector.tensor_mul(t2[:], sw[:], st[:])
                nc.vector.tensor_add(rot[:, m, :], t1[:], t2[:])

        # ---------------- V projection (token-major, frame-ordered) -------
        def v_proj():
          with ExitStack() as pctx:
            pspool = pctx.enter_context(
                tc.tile_pool(name="ps_v", bufs=4, space="PSUM"))
            evpool = pctx.enter_context(tc.tile_pool(name="ev_v", bufs=3))
            w_sb = wpool.tile([128, NH, DIM], dt.bfloat16, tag="w")
            nc.sync.dma_start(
                out=w_sb[:], in_=w_d["wvT"][:].rearrange("(kc p) n -> p kc n", p=128))
            # frame kf's rows are complete once chunks covering [kf*T,(kf+1)*T)
            # are evicted; issue that frame's AllGather right after.
            fr_done = {}
            for kf in range(F):
                last_ti = max(ti for ti, (ta, tb) in enumerate(TOKCH)
                              if ta < (kf + 1) * T)
                fr_done.setdefault(last_ti, []).append(kf)
            for ti, (ta, tb) in enumerate(TOKCH):
                tw = tb - ta
                for sl in range(NSL):
                    ps = pspool.tile([128, SLICE], dt.float32, tag="vps")
                    for kc in range(NH):
                        nc.tensor.matmul(ps[:tw, :], xT_sb[:, kc, ta:tb],
                                         w_sb[:, kc, sl * SLICE:(sl + 1) * SLICE],
                                         start=(kc == 0), stop=False)
                    nc.tensor.matmul(ps[:tw, :], ones_row[0:1, :tw],
                                     bv_sb[0:1, sl * SLICE:(sl + 1) * SLICE],
                                     start=False, stop=True)
                    vt = evpool.tile([128, SLICE], dt.bfloat16, tag="vev")
                    nc.scalar.activation(vt[:tw, :], ps[:tw, :], AF.Copy)
                    # split the eviction at frame boundaries (per-frame tiles)
                    for kf in range(F):
                        ia, ib = max(ta, kf * T), min(tb, (kf + 1) * T)
                        if ia >= ib:
                            continue
                        nc.sync.dma_start(
                            out=v_loc_f[kf][ia - kf * T:ib - kf * T,
                                            sl * SLICE:(sl + 1) * SLICE],
                            in_=vt[ia - ta:ib - ta, :])
                if ti in fr_done and use_collective:
                    for kf in fr_done[ti]:
                        nc.gpsimd.collective_compute(
                            "AllGather", mybir.AluOpType.bypass,
                            ins=[v_loc_f[kf][:]], outs=[v_all_f[kf][:]],
                            replica_groups=rg)

        # ---- phase order: K first (collective early), then V, then Q ----
        qk_proj("wkT", bk_sb, gk_sb, bkgk_sb, khat, r_k)
        rope(khat, krot, r_k, "k")
        for m in range(NH):
            nc.sync.dma_start(out=k_loc[m * 128:(m + 1) * 128, :], in_=krot[:, m, :])
        if use_collective:
            nc.gpsimd.collective_compute(
                "AllGather", mybir.AluOpType.bypass, ins=[k_loc[:]],
                outs=[k_all[:]], replica_groups=rg)
        v_proj()
        qk_proj("wqT", bq_sb, gq_sb, bqgq_sb, qhat, r_q)
        rope(qhat, qrot, r_q, "q")
        prep.close()  # free x / hats / krot / angles before attention

        # ---------------- attention ----------------
        # Per (head, key-frame): one K tile [128, FRAME] (strided gather over
        # cores) and one V tile [128, nch, 128] token-major.  Keys chunked
        # 12x128+tail.  Scores s^T [keys, q] into 2-chunk PSUM tiles
        # (seg-a, q in [q0, QA)) plus a shared tail bank (seg-b, q in
        # [QA, S), TPB chunk-slots per bank).  exp on ACT per PSUM tile.
        # o accumulated per head in 2 banks; z by ones-matmuls into 1 bank.
        actx = ExitStack()
        att_k = actx.enter_context(tc.tile_pool(name="att_k", bufs=3))
        att_v = actx.enter_context(tc.tile_pool(name="att_v", bufs=3))
        att_s = actx.enter_context(tc.tile_pool(name="att_s", bufs=2, space="PSUM"))
        att_st = actx.enter_context(tc.tile_pool(name="att_st", bufs=1, space="PSUM"))
        att_oa = actx.enter_context(tc.tile_pool(name="att_oa", bufs=2, space="PSUM"))
        att_ob = actx.enter_context(tc.tile_pool(name="att_ob", bufs=1, space="PSUM"))
        att_p = actx.enter_context(tc.tile_pool(name="att_p", bufs=2))
        att_t = actx.enter_context(tc.tile_pool(name="att_t", bufs=2))
        att_m = actx.enter_context(tc.tile_pool(name="att_m", bufs=2))

        NKC = len(KCH)
        assert q0_min == 0, "oT zero-fill for q < q0_min not implemented"
        for h in range(NH):
            o_a = att_o.tile([128, QA - q0_min], dt.float32, tag="oa", name="oa")
            o_b = (att_o.tile([128, TAILW], dt.float32, tag="ob", name="ob")
                   if TAILW else None)
            z_t = att_z.tile([128, BANK], dt.float32, tag="z", name="z")
            # z rows: row0 at partition 0 covers q [q0_min, QA); row1 at
            # partition 32 covers q [QA, S)

            for kf in range(F):
                q0 = T * first_qf[kf]
                kr_t = att_k.tile([128, NC * T], dt.bfloat16, tag="kr")
                nc.sync.dma_start(
                    out=kr_t[:].rearrange("p (c t) -> p c t", c=NC),
                    in_=k_all[:, kf * T:(kf + 1) * T]
                    .rearrange("(c m p) t -> p c m t", c=NC, p=128)[:, :, h, :])
                v_t = att_v.tile([128, NKC, 128], dt.bfloat16, tag="vt")
                nfull = FRAME // 128
                v_view = v_all_f[kf][:, h * 128:(h + 1) * 128]
                nc.sync.dma_start(
                    out=v_t[:, :nfull, :],
                    in_=v_view[:nfull * 128, :].rearrange("(j p) n -> p j n", p=128))
                if FRAME % 128:
                    nc.sync.dma_start(
                        out=v_t[:FRAME % 128, nfull:nfull + 1, :],
                        in_=v_view[nfull * 128:, :].rearrange(
                            "(j p) n -> p j n", p=FRAME % 128))

                # per-(head, kf) p buffer: all chunks' probabilities, so the
                # softmax denominator can be tree-reduced on DVE
                p_kf = att_p.tile([128, NKC, S], dt.bfloat16, tag="p")
                kw_tail = KCH[-1][1] - KCH[-1][0]
                if kw_tail < 128:
                    # zero the tail chunk's slot (engine base-partition must
                    # be 32-aligned, so clear all rows; exp then overwrites
                    # the valid ones) so the z tree can include it blindly
                    nc.vector.memset(p_kf[:, NKC - 1, q0:S], 0.0)

                # seg-a (q in [q0, QA)): chunk pairs through 2-bank tiles
                ci = 0
                while ci < NKC:
                    pair = [c for c in (ci, ci + 1) if c < NKC]
                    s_t = att_s.tile([128, 2, BANK], dt.float32, tag="s")
                    for i, c in enumerate(pair):
                        ka, kb = KCH[c]
                        kw = kb - ka
                        nc.tensor.matmul(s_t[:kw, i, :QA - q0],
                                         kr_t[:, ka:kb],
                                         qrot[:, h, q0:QA],
                                         start=True, stop=True)
                    nfu = sum(1 for c in pair if KCH[c][1] - KCH[c][0] == 128)
                    if nfu:
                        nc.scalar.activation(
                            p_kf[:, ci:ci + nfu, q0:QA],
                            s_t[:, :nfu, :QA - q0],
                            AF.Exp, scale=inv_sqrt_d)
                    if nfu < len(pair):  # tail chunk: only its valid rows
                        nc.scalar.activation(
                            p_kf[:kw_tail, ci + nfu:ci + len(pair), q0:QA],
                            s_t[:kw_tail, nfu:len(pair), :QA - q0],
                            AF.Exp, scale=inv_sqrt_d)
                    for i, c in enumerate(pair):
                        ka, kb = KCH[c]
                        kw = kb - ka
                        nc.tensor.matmul(o_a[:, q0 - q0_min:QA - q0_min],
                                         v_t[:kw, c, :],
                                         p_kf[:kw, c, q0:QA],
                                         start=(kf == 0 and c == 0),
                                         stop=(kf == F - 1 and c == NKC - 1))
                    ci += 2

                # seg-b (tail q columns) in TPB-chunk groups
                if TAILW:
                    ci = 0
                    while ci < NKC:
                        grp = list(range(ci, min(ci + TPB, NKC)))
                        st_t = att_st.tile([128, TPB, TAILW], dt.float32, tag="st")
                        for i, c in enumerate(grp):
                            ka, kb = KCH[c]
                            kw = kb - ka
                            nc.tensor.matmul(st_t[:kw, i, :],
                                             kr_t[:, ka:kb],
                                             qrot[:, h, QA:S],
                                             start=True, stop=True)
                        nfu = sum(1 for c in grp
                                  if KCH[c][1] - KCH[c][0] == 128)
                        if nfu:
                            nc.scalar.activation(
                                p_kf[:, ci:ci + nfu, QA:S],
                                st_t[:, :nfu, :],
                                AF.Exp, scale=inv_sqrt_d)
                        if nfu < len(grp):
                            nc.scalar.activation(
                                p_kf[:kw_tail, ci + nfu:ci + len(grp), QA:S],
                                st_t[:kw_tail, nfu:len(grp), :],
                                AF.Exp, scale=inv_sqrt_d)
                        for i, c in enumerate(grp):
                            ka, kb = KCH[c]
                            kw = kb - ka
                            nc.tensor.matmul(o_b[:, :], v_t[:kw, c, :],
                                             p_kf[:kw, c, QA:S],
                                             start=(kf == 0 and c == 0),
                                             stop=(kf == F - 1 and c == NKC - 1))
                        ci += TPB

                # softmax denominator: chunk-axis tree on DVE (level 1 in
                # bf16, the rest fp32), then one ones-matmul per q segment
                # reduces the 128 key partitions into the z bank.
                nh_ = NKC // 2
                t1 = att_t.tile([128, nh_, S], dt.bfloat16, tag="t1")
                nc.vector.tensor_add(t1[:, :, q0:S], p_kf[:, 0:nh_, q0:S],
                                     p_kf[:, nh_:2 * nh_, q0:S])
                n2 = nh_ // 2
                t2 = att_t.tile([128, n2, S], dt.float32, tag="t2")
                nc.vector.tensor_add(t2[:, :, q0:S], t1[:, 0:n2, q0:S],
                                     t1[:, n2:2 * n2, q0:S])
                for sl in range(1, n2):
                    nc.vector.tensor_add(t2[:, 0, q0:S], t2[:, 0, q0:S],
                                         t2[:, sl, q0:S])
                if nh_ % 2:  # odd level-1 slot
                    nc.vector.tensor_add(t2[:, 0, q0:S], t2[:, 0, q0:S],
                                         t1[:, nh_ - 1, q0:S])
                if NKC % 2:  # odd chunk (never paired at level 1)
                    nc.vector.tensor_add(t2[:, 0, q0:S], t2[:, 0, q0:S],
                                         p_kf[:, NKC - 1, q0:S])
                nc.tensor.matmul(z_t[0:1, q0 - q0_min:QA - q0_min],
                                 ones_f32[:, :], t2[:, 0, q0:QA],
                                 start=(kf == 0), stop=(kf == F - 1))
                if TAILW:
                    nc.tensor.matmul(z_t[32:33, :TAILW],
                                     ones_f32[:, :], t2[:, 0, QA:S],
                                     start=(kf == 0), stop=(kf == F - 1))

            # 1/Z and eviction for head h
            z_sb = att_m.tile([1, S], dt.float32, tag="zsb", name="zsb")
            nc.scalar.activation(z_sb[0:1, q0_min:QA],
                                 z_t[0:1, :QA - q0_min], AF.Copy)
            if TAILW:
                nc.vector.tensor_copy(z_sb[0:1, QA:S], z_t[32:33, :TAILW])
            nc.vector.reciprocal(z_sb[0:1, :], z_sb[0:1, :])
            z_dram = dram.tile([1, S], dt.float32, tag="zdram", bufs=2,
                               name="zdram")
            nc.sync.dma_start(out=z_dram[0:1, :], in_=z_sb[0:1, :])
            izb = att_m.tile([128, S], dt.float32, tag="izb", name="izb")
            nc.sync.dma_start(
                out=izb[:],
                in_=bass.AP(tensor=z_dram.tensor, offset=z_dram[0:1, :].offset,
                            ap=[[0, 128]] + z_dram[0:1, :].ap[1:]))
            nc.vector.tensor_mul(oT_sb[:, h, q0_min:QA],
                                 o_a[:, :], izb[:, q0_min:QA])
            if TAILW:
                nc.vector.tensor_mul(oT_sb[:, h, QA:S], o_b[:, :],
                                     izb[:, QA:S])

        actx.close()  # release attention PSUM banks before the O-projection

        # ---------------- O projection ----------------
        wopool = ctx.enter_context(tc.tile_pool(name="w_o", bufs=3))
        pspool = ctx.enter_context(
            tc.tile_pool(name="ps_o", bufs=len(TOKCH) + 1, space="PSUM"))
        evpool = ctx.enter_context(tc.tile_pool(name="ev_o", bufs=3))
        for sl in range(NSL):
            ps = {}
            for ti in range(len(TOKCH)):
                ps[ti] = pspool.tile([128, SLICE], dt.float32, tag="ops", name=f"ops{ti}")
            for m in range(NH):
                wt = wopool.tile([128, SLICE], dt.bfloat16, tag="wo")
                nc.sync.dma_start(
                    out=wt[:],
                    in_=w_d["woT"][m * 128:(m + 1) * 128,
                                   sl * SLICE:(sl + 1) * SLICE])
                for ti, (ta, tb) in enumerate(TOKCH):
                    nc.tensor.matmul(ps[ti][:tb - ta, :], oT_sb[:, m, ta:tb],
                                     wt[:], start=(m == 0), stop=(m == NH - 1))
            for ti, (ta, tb) in enumerate(TOKCH):
                tw = tb - ta
                ot = evpool.tile([128, SLICE], dt.float32, tag="oev")
                nc.vector.tensor_add(ot[:tw, :], ps[ti][:tw, :],
                                     bo_bc[:tw, sl * SLICE:(sl + 1) * SLICE])
                nc.sync.dma_start(
                    out=out_d[ta:tb, sl * SLICE:(sl + 1) * SLICE],
                    in_=ot[:tw, :])

    if cap_waits:
        _cap_sync_waits(nc, mybir)
    _BUILD_CACHE[key] = nc
    return nc


def _cap_sync_waits(nc, mybir, cap=1):
    """Walrus engine-instruction structs only have a limited number of sync
    wait slots.  Hoist excess waits onto InstNoOp carriers placed immediately
    before the instruction on the same engine stream."""
    exempt = (mybir.InstNoOp, mybir.InstEventSemaphore,
              mybir.InstAllEngineBarrier)
    for f in nc.m.functions:
        for bb in f.blocks:
            out = []
            changed = False
            for inst in bb.instructions:
                si = inst.sync_info
                if (si is None or len(si.on_wait) <= cap
                        or isinstance(inst, exempt)):
                    out.append(inst)
                    continue
                waits = list(si.on_wait)
                keep, excess = waits[:cap], waits[cap:]
                while excess:
                    batch, excess = excess[:cap], excess[cap:]
                    out.append(mybir.InstNoOp(
                        name=f"{inst.name}-w{len(out)}",
                        engine=inst.engine,
                        bass_nofuse=True,
                        sync_info=mybir.SyncInfo(on_wait=batch, on_update=[]),
                    ))
                inst.sync_info = mybir.SyncInfo(on_wait=keep,
                                                on_update=list(si.on_update))
                out.append(inst)
                changed = True
            if changed:
                bb.instructions = out


# ---------------------------------------------------------------------------
# host side
# ---------------------------------------------------------------------------
def _perm(NH):
    p = np.empty(NH * D, np.int64)
    for hh in range(NH):
        base = hh * D
        for j in range(D // 2):
            p[base + j] = base + 2 * j
            p[base + D // 2 + j] = base + 2 * j + 1
    return p


def _host_inputs(x, freqs, Wq, bq, Wk, bk, Wv, bv, Wo, bo, gq, gk,
                 f, h, w, num_heads, local_attn_size, sink_size, start_frame):
    NH = num_heads
    DIM = NH * D
    FRAME = h * w
    assert FRAME % NC == 0
    T = FRAME // NC
    S = f * T
    perm = _perm(NH)

    def bf(a):
        return np.ascontiguousarray(a, dtype=np.float32).astype(BF16)

    wqT = bf(Wq[perm].T)
    wkT = bf(Wk[perm].T)
    wvT = bf(Wv.T)
    woT = bf(Wo.T)
    def chunkmajor(a):
        return np.asarray(a, np.float32)[perm].reshape(NH, D).T
    bias_pack = np.ascontiguousarray(np.concatenate(
        [chunkmajor(bq), chunkmajor(gq), chunkmajor(bq) * chunkmajor(gq),
         chunkmajor(bk), chunkmajor(gk), chunkmajor(bk) * chunkmajor(gk)],
        axis=1), np.float32)
    bv_r = bf(bv.reshape(1, DIM))
    bo_r = np.ascontiguousarray(bo.reshape(1, DIM), np.float32)

    c = D // 2
    c1 = c // 3
    c0 = c - 2 * c1
    freqs = np.asarray(freqs, np.float32)

    in_maps = []
    tok_idx = []
    for core in range(NC):
        idx = np.concatenate(
            [fr * FRAME + T * core + np.arange(T) for fr in range(f)])
        tok_idx.append(idx)
        xT = bf(np.asarray(x[0], np.float32)[idx].T)
        fr = idx // FRAME
        rem = idx % FRAME
        hh_i = rem // w
        ww_i = rem % w
        ang = np.empty((c, S), np.float32)
        ang[:c0, :] = freqs[start_frame + fr][:, :c0].T
        ang[c0:c0 + c1, :] = freqs[hh_i][:, c0:c0 + c1].T
        ang[c0 + c1:, :] = freqs[ww_i][:, c0 + c1:c].T
        def wrap(a):
            a = np.asarray(a, np.float64)
            return (a - 2 * np.pi * np.round(a / (2 * np.pi))).astype(np.float32)
        # top half encodes -sin via the (ang + pi) phase shift
        angS = np.ascontiguousarray(
            np.concatenate([wrap(ang + np.pi), wrap(ang)], 0), np.float32)
        angC = np.ascontiguousarray(
            np.concatenate([wrap(ang + np.pi / 2), wrap(ang + np.pi / 2)], 0),
            np.float32)
        in_maps.append({
            "xT": xT, "wqT": wqT, "wkT": wkT, "wvT": wvT, "woT": woT,
            "bias_pack": bias_pack,
            "bv_r": bv_r, "bo_r": bo_r, "angS": angS, "angC": angC,
        })
    return in_maps, tok_idx, T, S


def _allowed(f, local_attn_size, sink_size):
    return [
        [kf for kf in range(f)
         if kf <= qf and (qf - kf < local_attn_size or kf < sink_size)]
        for qf in range(f)
    ]


def kernel(x, freqs, Wq, bq, Wk, bk, Wv, bv, Wo, bo, gq, gk,
           f, h, w, num_heads, local_attn_size, sink_size, start_frame,
           _trace=False):
    from concourse.bass_utils import run_bass_kernel_spmd

    f = int(f); h = int(h); w = int(w)
    num_heads = int(num_heads)
    local_attn_size = int(local_attn_size)
    sink_size = int(sink_size)
    start_frame = int(start_frame)

    x = np.asarray(x)
    B, L, DIM = x.shape
    assert B == 1 and DIM == num_heads * D

    allowed = _allowed(f, local_attn_size, sink_size)
    in_maps, tok_idx, T, S = _host_inputs(
        x, freqs, Wq, bq, Wk, bk, Wv, bv, Wo, bo, gq, gk,
        f, h, w, num_heads, local_attn_size, sink_size, start_frame)
    nc = build_program(num_heads, f, T, allowed)
    res = run_bass_kernel_spmd(nc, in_maps, core_ids=list(range(NC)),
                               trace=_trace)
    out = np.empty((1, L, DIM), np.float32)
    for core in range(NC):
        out[0, tok_idx[core]] = res.results[core]["out"]
    if _trace:
        kernel._last_results = res
    return out


# revision 21
# speedup vs baseline: 2.1254x; 1.6662x over previous
"""Trainium2 Bass kernel for CausalWanSelfAttention (frame-causal windowed
attention with QK-RMSNorm + RoPE), sharded over 8 NeuronCores.

Sharding: each core owns T = (h*w)/8 tokens of every frame (frame-balanced
interleave).  Each core computes Q/K/V projections + RMSNorm + RoPE for its
own tokens, K/V are exchanged with AllGathers (K whole; V per-frame so the
gathered V is frame-contiguous), attention + O-projection are computed
locally for the core's query tokens.

Device layouts:
  - q/k feature-major [ch, tok] (channels on partitions), with each head's
    128 channels permuted to [re(0..63) | im(0..63)] so RoPE works on
    contiguous partition blocks (permutation is folded into Wq/Wk on host).
  - v token-major [tok, ch] (natural channel order).
  - attention works on frame-contiguous key tiles: per (head, key-frame) a
    single strided DMA assembles K^T [128ch, FRAME] and V [FRAME, 128ch]
    from the gathered buffers; keys are chunked 12x128+tail.
  - scores computed as s^T [keys, q] -> exp on ACT over multi-chunk PSUM
    tiles (2-chunk main segments + shared 7-chunk tail bank).
  - softmax denominator Z accumulated by ones-matmuls into a dedicated
    PSUM bank (rows at partitions 0/32); per-head 1/Z applied by DVE
    during o-PSUM eviction with a partition-broadcast tile.
  - RMSNorm scale r (per token) is folded into the RoPE cos/sin tables;
    per-channel gain g and bias b are folded into the ACT eviction.
"""

import math
import sys
from contextlib import ExitStack

import numpy as np

if "/opt/trn_rl_repo" not in sys.path:
    sys.path.insert(0, "/opt/trn_rl_repo")

import ml_dtypes

BF16 = ml_dtypes.bfloat16
NC = 8  # cores
D = 128  # head dim
EPS = 1e-6


def _chunks(n, width=128):
    return [(g * width, min(n, (g + 1) * width))
            for g in range((n + width - 1) // width)]


# ---------------------------------------------------------------------------
# device program
# ---------------------------------------------------------------------------
_BUILD_CACHE = {}


def build_program(NH, F, T, allowed_kf, cap_waits=True, use_collective=True):
    """Build the SPMD Bass program (identical on all 8 cores).

    NH: number of heads; F: frames; T: tokens per (core, frame);
    allowed_kf[qf] = list of key frames query-frame qf may attend to
    (must make, for each kf, the attending q-set a contiguous suffix of
    frames -- true for causal masks).
    """
    key = (NH, F, T, tuple(tuple(a) for a in allowed_kf), cap_waits,
           use_collective)
    if key in _BUILD_CACHE:
        return _BUILD_CACHE[key]

    import concourse.bass as bass
    import concourse.mybir as mybir
    import concourse.tile as tile
    from concourse.mybir import ActivationFunctionType as AF

    dt = mybir.dt
    DIM = NH * D
    S = F * T              # tokens per core
    FRAME = NC * T         # tokens per frame (= keys per frame)
    H0 = (S + 1) // 2      # token halves for the q/k projections
    SLICE = min(512, DIM)  # out-channel slice for v/o projections
    NSL = DIM // SLICE
    TOKCH = _chunks(S, 128)  # token chunks for v/o projections
    KCH = _chunks(FRAME, 128)  # key chunks within a frame (frame-contiguous)
    BANK = 512             # fp32 elements per PSUM bank

    # for each key frame kf: the first query frame that attends to it
    first_qf = {}
    for kf in range(F):
        qs = [qf for qf in range(F) if kf in allowed_kf[qf]]
        assert qs, f"key frame {kf} unused"
        assert qs == list(range(qs[0], F)), "non-suffix q-set unsupported"
        first_qf[kf] = qs[0]
    q0_min = min(T * first_qf[kf] for kf in range(F))

    # q split: seg-a = [q0(kf), QA), seg-b = [QA, S) shared-tail
    QA = min(BANK, S)
    TAILW = S - QA                        # 73 for S=585
    assert TAILW * 4 <= 2048 or TAILW == 0
    TPB = max(1, (2048 // (TAILW * 4)) if TAILW else 1)  # tail slots per bank

    nc = bass.Bass()

    # ---------------- I/O ----------------
    xT_d = nc.dram_tensor("xT", [DIM, S], dt.bfloat16, kind="ExternalInput")
    w_d = {}
    for nm in ("wqT", "wkT", "wvT", "woT"):
        w_d[nm] = nc.dram_tensor(nm, [DIM, DIM], dt.bfloat16, kind="ExternalInput")
    # packed per-channel affine constants: bq|gq|bq*gq|bk|gk|bk*gk
    bias_d = nc.dram_tensor("bias_pack", [128, 6 * NH], dt.float32,
                            kind="ExternalInput")
    bv_d = nc.dram_tensor("bv_r", [1, DIM], dt.bfloat16, kind="ExternalInput")
    bo_d = nc.dram_tensor("bo_r", [1, DIM], dt.float32, kind="ExternalInput")
    angS_d = nc.dram_tensor("angS", [128, S], dt.float32, kind="ExternalInput")
    angC_d = nc.dram_tensor("angC", [128, S], dt.float32, kind="ExternalInput")
    out_d = nc.dram_tensor("out", [S, DIM], dt.float32, kind="ExternalOutput")

    rg = [list(range(NC))]
    inv_sqrt_d = 1.0 / math.sqrt(D)

    with tile.TileContext(nc) as tc, ExitStack() as ctx:
        dram = ctx.enter_context(tc.tile_pool(name="dram", bufs=1, space="DRAM"))
        k_loc = dram.tile([DIM, S], dt.bfloat16)
        # per-frame V buffers: separate tiles so the per-frame AllGathers
        # never false-share (whole-tile dep tracking) with later writes/reads
        v_loc_f = [dram.tile([T, DIM], dt.bfloat16, name=f"vloc{kf}")
                   for kf in range(F)]
        k_all = dram.tile([NC * DIM, S], dt.bfloat16, addr_space="Shared")
        v_all_f = [dram.tile([FRAME, DIM], dt.bfloat16, addr_space="Shared",
                             name=f"vall{kf}")
                   for kf in range(F)]

        const = ctx.enter_context(tc.tile_pool(name="const", bufs=1))
        resid = ctx.enter_context(tc.tile_pool(name="resid", bufs=1))

        ones_key = const.tile([128, 1], dt.bfloat16)
        nc.vector.memset(ones_key, 1.0)
        ones_f32 = const.tile([128, 1], dt.float32)
        nc.vector.memset(ones_f32, 1.0)
        ones_row = const.tile([1, 128], dt.bfloat16)
        nc.vector.memset(ones_row, 1.0)
        eps_t = const.tile([128, 1], dt.float32)
        nc.vector.memset(eps_t, EPS)

        # constant / bias tiles (one DMA for the packed affine constants)
        bias_sb = const.tile([128, 6 * NH], dt.float32)
        nc.sync.dma_start(out=bias_sb[:], in_=bias_d[:])
        bq_sb = bias_sb[:, 0 * NH:1 * NH]
        gq_sb = bias_sb[:, 1 * NH:2 * NH]
        bqgq_sb = bias_sb[:, 2 * NH:3 * NH]
        bk_sb = bias_sb[:, 3 * NH:4 * NH]
        gk_sb = bias_sb[:, 4 * NH:5 * NH]
        bkgk_sb = bias_sb[:, 5 * NH:6 * NH]
        bv_sb = const.tile([1, DIM], dt.bfloat16)
        nc.sync.dma_start(out=bv_sb[:], in_=bv_d[:])
        bo_bc = const.tile([128, DIM], dt.float32)
        nc.sync.dma_start(
            out=bo_bc[:],
            in_=bass.AP(tensor=bo_d[:].tensor, offset=bo_d[:].offset,
                        ap=[[0, 128]] + bo_d[:].ap[1:]),
        )

        # persistent across phases: rotated q and attention output
        qrot = resid.tile([128, NH, S], dt.bfloat16)
        oT_sb = resid.tile([128, NH, S], dt.bfloat16)

        # prep-phase tensors (freed before attention)
        prep = ExitStack()
        prepp = prep.enter_context(tc.tile_pool(name="prep", bufs=1))

        # x (feature-major), resident through the projections
        xT_sb = prepp.tile([128, NH, S], dt.bfloat16)
        nc.sync.dma_start(out=xT_sb[:], in_=xT_d[:].rearrange("(m p) s -> p m s", p=128))

        # raw RoPE sin/cos (shared q/k)
        angS_sb = prepp.tile([128, S], dt.float32)
        angC_sb = prepp.tile([128, S], dt.float32)
        nc.sync.dma_start(out=angS_sb[:], in_=angS_d[:])
        nc.sync.dma_start(out=angC_sb[:], in_=angC_d[:])
        # angles arrive host-canonicalized to [-pi, pi] (ACT Sin table range)
        sin_raw = prepp.tile([128, S], dt.float32)
        cos_raw = prepp.tile([128, S], dt.float32)
        nc.scalar.activation(sin_raw[:], angS_sb[:], AF.Sin)
        nc.scalar.activation(cos_raw[:], angC_sb[:], AF.Sin)

        qhat = prepp.tile([128, NH, S], dt.bfloat16)
        khat = prepp.tile([128, NH, S], dt.bfloat16)
        krot = prepp.tile([128, NH, S], dt.bfloat16)
        r_q = prepp.tile([1, S], dt.float32)
        r_k = prepp.tile([1, S], dt.float32)

        halves = [(0, H0), (H0, S)] if S > H0 else [(0, S)]

        wpool = prep.enter_context(tc.tile_pool(name="w_qkv", bufs=2))

        # ---------------- Q/K projections + RMS stats ----------------
        def qk_proj(wname, bias_sb, gain_sb, bg_sb, hat, r_sb):
          with ExitStack() as pctx:
            pspool = pctx.enter_context(
                tc.tile_pool(name=f"ps_{wname}", bufs=4, space="PSUM"))
            sspool = pctx.enter_context(
                tc.tile_pool(name=f"ss_{wname}", bufs=2, space="PSUM"))
            evpool = pctx.enter_context(tc.tile_pool(name=f"ev_{wname}", bufs=3))
            w_sb = wpool.tile([128, NH, DIM], dt.bfloat16, tag="w")
            nc.sync.dma_start(
                out=w_sb[:], in_=w_d[wname][:].rearrange("(kc p) n -> p kc n", p=128))
            ss_ps = {}
            for hi, (ha, hb) in enumerate(halves):
                ss_ps[hi] = sspool.tile([1, hb - ha], dt.float32, tag="ss", name=f"ss{hi}")
            for m in range(NH):
                ps = {}
                for hi, (ha, hb) in enumerate(halves):
                    ps[hi] = pspool.tile([128, hb - ha], dt.float32, tag="ps", name=f"ps{hi}")
                for kc in range(NH):
                    for hi, (ha, hb) in enumerate(halves):
                        nc.tensor.matmul(ps[hi][:, :hb - ha],
                                         w_sb[:, kc, m * 128:(m + 1) * 128],
                                         xT_sb[:, kc, ha:hb],
                                         start=(kc == 0), stop=(kc == NH - 1))
                for hi, (ha, hb) in enumerate(halves):
                    hw_ = hb - ha
                    sq = evpool.tile([128, H0], dt.bfloat16, tag="sq")
                    # (q + b)^2
                    nc.scalar.activation(sq[:, :hw_], ps[hi][:, :hw_], AF.Square,
                                         bias=bias_sb[:, m:m + 1])
                    # qhat = (q + b) * g = q*g + b*g
                    nc.scalar.activation(hat[:, m, ha:hb], ps[hi][:, :hw_],
                                         AF.Identity, bias=bg_sb[:, m:m + 1],
                                         scale=gain_sb[:, m:m + 1])
                    nc.tensor.matmul(ss_ps[hi][0:1, :hw_], ones_key[:],
                                     sq[:, :hw_],
                                     start=(m == 0), stop=(m == NH - 1))
            for hi, (ha, hb) in enumerate(halves):
                hw_ = hb - ha
                rt = evpool.tile([1, H0], dt.float32, tag="rt")
                # sqrt(mean(q^2) + eps)
                nc.scalar.activation(rt[0:1, :hw_], ss_ps[hi][0:1, :hw_], AF.Sqrt,
                                     bias=eps_t[0:1, :], scale=1.0 / DIM)
                nc.vector.reciprocal(r_sb[0:1, ha:hb], rt[0:1, :hw_])

        # ---------------- RoPE ----------------
        def rope(hat, rot, r_sb, tag):
          with ExitStack() as pctx:
            rp = pctx.enter_context(tc.tile_pool(name=f"rope_{tag}", bufs=3))
            r_dram = dram.tile([1, S], dt.float32, name=f"rdram_{tag}")
            nc.sync.dma_start(out=r_dram[:], in_=r_sb[0:1, :])
            rb = prepp.tile([128, S], dt.float32, name=f"rb_{tag}")
            nc.sync.dma_start(
                out=rb[:],
                in_=bass.AP(tensor=r_dram.tensor, offset=r_dram[0:1, :].offset,
                            ap=[[0, 128]] + r_dram[0:1, :].ap[1:]))
            ct = prepp.tile([128, S], dt.bfloat16, name=f"cos_{tag}")
            st = prepp.tile([128, S], dt.bfloat16, name=f"sin_{tag}")
            nc.vector.tensor_mul(ct[:], cos_raw[:], rb[:])
            nc.vector.tensor_mul(st[:], sin_raw[:], rb[:])
            for m in range(NH):
                sw = rp.tile([128, S], dt.bfloat16, tag="sw")
                nc.sync.dma_start(out=sw[0:64, :], in_=hat[64:128, m, :])
                nc.sync.dma_start(out=sw[64:128, :], in_=hat[0:64, m, :])
                t1 = rp.tile([128, S], dt.bfloat16, tag="t1")
                t2 = rp.tile([128, S], dt.bfloat16, tag="t2")
                nc.vector.tensor_mul(t1[:], hat[:, m, :], ct[:])
                nc.vector.tensor_mul(t2[:], sw[:], st[:])
                nc.vector.tensor_add(rot[:, m, :], t1[:], t2[:])

        # ---------------- V projection (token-major, frame-ordered) -------
        def v_proj():
          with ExitStack() as pctx:
            pspool = pctx.enter_context(
                tc.tile_pool(name="ps_v", bufs=4, space="PSUM"))
            evpool = pctx.enter_context(tc.tile_pool(name="ev_v", bufs=3))
            w_sb = wpool.tile([128, NH, DIM], dt.bfloat16, tag="w")
            nc.sync.dma_start(
                out=w_sb[:], in_=w_d["wvT"][:].rearrange("(kc p) n -> p kc n", p=128))
            # frame kf's rows are complete once chunks covering [kf*T,(kf+1)*T)
            # are evicted; issue that frame's AllGather right after.
            fr_done = {}
            for kf in range(F):
                last_ti = max(ti for ti, (ta, tb) in enumerate(TOKCH)
                              if ta < (kf + 1) * T)
                fr_done.setdefault(last_ti, []).append(kf)
            for ti, (ta, tb) in enumerate(TOKCH):
                tw = tb - ta
                for sl in range(NSL):
                    ps = pspool.tile([128, SLICE], dt.float32, tag="vps")
                    for kc in range(NH):
                        nc.tensor.matmul(ps[:tw, :], xT_sb[:, kc, ta:tb],
                                         w_sb[:, kc, sl * SLICE:(sl + 1) * SLICE],
                                         start=(kc == 0), stop=False)
                    nc.tensor.matmul(ps[:tw, :], ones_row[0:1, :tw],
                                     bv_sb[0:1, sl * SLICE:(sl + 1) * SLICE],
                                     start=False, stop=True)
                    vt = evpool.tile([128, SLICE], dt.bfloat16, tag="vev")
                    nc.scalar.activation(vt[:tw, :], ps[:tw, :], AF.Copy)
                    # split the eviction at frame boundaries (per-frame tiles)
                    for kf in range(F):
                        ia, ib = max(ta, kf * T), min(tb, (kf + 1) * T)
                        if ia >= ib:
                            continue
                        nc.sync.dma_start(
                            out=v_loc_f[kf][ia - kf * T:ib - kf * T,
                                            sl * SLICE:(sl + 1) * SLICE],
                            in_=vt[ia - ta:ib - ta, :])
                if ti in fr_done and use_collective:
                    for kf in fr_done[ti]:
                        nc.gpsimd.collective_compute(
                            "AllGather", mybir.AluOpType.bypass,
                            ins=[v_loc_f[kf][:]], outs=[v_all_f[kf][:]],
                            replica_groups=rg)

        # ---- phase order: K first (collective early), then V, then Q ----
        qk_proj("wkT", bk_sb, gk_sb, bkgk_sb, khat, r_k)
        rope(khat, krot, r_k, "k")
        for m in range(NH):
            nc.sync.dma_start(out=k_loc[m * 128:(m + 1) * 128, :], in_=krot[:, m, :])
        if use_collective:
            nc.gpsimd.collective_compute(
                "AllGather", mybir.AluOpType.bypass, ins=[k_loc[:]],
                outs=[k_all[:]], replica_groups=rg)
        v_proj()
        qk_proj("wqT", bq_sb, gq_sb, bqgq_sb, qhat, r_q)
        rope(qhat, qrot, r_q, "q")
        prep.close()  # free x / hats / krot / angles before attention

        # ---------------- attention ----------------
        # Per (head, key-frame): one K tile [128, FRAME] (strided gather over
        # cores) and one V tile [128, nch, 128] token-major.  Keys chunked
        # 12x128+tail.  Scores s^T [keys, q] into 2-chunk PSUM tiles
        # (seg-a, q in [q0, QA)) plus a shared tail bank (seg-b, q in
        # [QA, S), TPB chunk-slots per bank).  exp on ACT per PSUM tile.
        # o accumulated per head in 2 banks; z by ones-matmuls into 1 bank.
        actx = ExitStack()
        att_k = actx.enter_context(tc.tile_pool(name="att_k", bufs=3))
        att_v = actx.enter_context(tc.tile_pool(name="att_v", bufs=3))
        att_s = actx.enter_context(tc.tile_pool(name="att_s", bufs=2, space="PSUM"))
        att_st = actx.enter_context(tc.tile_pool(name="att_st", bufs=1, space="PSUM"))
        att_oa = actx.enter_context(tc.tile_pool(name="att_oa", bufs=2, space="PSUM"))
        att_ob = actx.enter_context(tc.tile_pool(name="att_ob", bufs=1, space="PSUM"))
        att_p = actx.enter_context(tc.tile_pool(name="att_p", bufs=2))
        att_t = actx.enter_context(tc.tile_pool(name="att_t", bufs=2))
        att_m = actx.enter_context(tc.tile_pool(name="att_m", bufs=2))

        NKC = len(KCH)
        assert q0_min == 0, "oT zero-fill for q < q0_min not implemented"
        # z lives in the o_b bank's spare columns: row z0 (partition 0)
        # covers q [q0_min, ZS), row z1 (partition 32) covers [ZS, S)
        ZS = (S + q0_min + 1) // 2
        assert TAILW + max(ZS - q0_min, S - ZS) <= BANK
        zrows = [(0, q0_min, ZS), (32, ZS, S)]

        def z_writers(lo, hi):
            return [kf for kf in range(F) if max(T * first_qf[kf], lo) < hi]

        for h in range(NH):
            o_a = att_oa.tile([128, QA - q0_min], dt.float32, tag="oa", name="oa")
            o_b = att_ob.tile([128, BANK], dt.float32, tag="ob", name="ob")

            for kf in range(F):
                q0 = T * first_qf[kf]
                kr_t = att_k.tile([128, NC * T], dt.bfloat16, tag="kr")
                nc.sync.dma_start(
                    out=kr_t[:].rearrange("p (c t) -> p c t", c=NC),
                    in_=k_all[:, kf * T:(kf + 1) * T]
                    .rearrange("(c m p) t -> p c m t", c=NC, p=128)[:, :, h, :])
                v_t = att_v.tile([128, NKC, 128], dt.bfloat16, tag="vt")
                nfull = FRAME // 128
                v_view = v_all_f[kf][:, h * 128:(h + 1) * 128]
                nc.sync.dma_start(
                    out=v_t[:, :nfull, :],
                    in_=v_view[:nfull * 128, :].rearrange("(j p) n -> p j n", p=128))
                if FRAME % 128:
                    nc.sync.dma_start(
                        out=v_t[:FRAME % 128, nfull:nfull + 1, :],
                        in_=v_view[nfull * 128:, :].rearrange(
                            "(j p) n -> p j n", p=FRAME % 128))

                # per-(head, kf) p buffer: all chunks' probabilities, so the
                # softmax denominator can be tree-reduced on DVE
                p_kf = att_p.tile([128, NKC, S], dt.bfloat16, tag="p")
                kw_tail = KCH[-1][1] - KCH[-1][0]
                if kw_tail < 128:
                    # zero the tail chunk's slot (engine base-partition must
                    # be 32-aligned, so clear all rows; exp then overwrites
                    # the valid ones) so the z tree can include it blindly
                    nc.vector.memset(p_kf[:, NKC - 1, q0:S], 0.0)

                # seg-a (q in [q0, QA)) through 2-bank pair tiles; when the
                # chunk's full q extent fits one bank slot, fold seg-b in
                # (one score MM extra, but a single exp covers [q0, S))
                fold_b = bool(TAILW) and (QA - q0 + TAILW <= BANK)
                qhi = S if fold_b else QA
                ci = 0
                while ci < NKC:
                    pair = [c for c in (ci, ci + 1) if c < NKC]
                    s_t = att_s.tile([128, 2, BANK], dt.float32, tag="s")
                    for i, c in enumerate(pair):
                        ka, kb = KCH[c]
                        kw = kb - ka
                        nc.tensor.matmul(s_t[:kw, i, :QA - q0],
                                         kr_t[:, ka:kb],
                                         qrot[:, h, q0:QA],
                                         start=True, stop=True)
                        if fold_b:
                            nc.tensor.matmul(s_t[:kw, i, QA - q0:QA - q0 + TAILW],
                                             kr_t[:, ka:kb],
                                             qrot[:, h, QA:S],
                                             start=True, stop=True)
                    nfu = sum(1 for c in pair if KCH[c][1] - KCH[c][0] == 128)
                    if nfu:
                        nc.scalar.activation(
                            p_kf[:, ci:ci + nfu, q0:qhi],
                            s_t[:, :nfu, :qhi - q0],
                            AF.Exp, scale=inv_sqrt_d)
                    if nfu < len(pair):  # tail chunk: only its valid rows
                        nc.scalar.activation(
                            p_kf[:kw_tail, ci + nfu:ci + len(pair), q0:qhi],
                            s_t[:kw_tail, nfu:len(pair), :qhi - q0],
                            AF.Exp, scale=inv_sqrt_d)
                    for i, c in enumerate(pair):
                        ka, kb = KCH[c]
                        kw = kb - ka
                        first = (kf == 0 and c == 0)
                        last = (kf == F - 1 and c == NKC - 1)
                        nc.tensor.matmul(o_a[:, q0 - q0_min:QA - q0_min],
                                         v_t[:kw, c, :],
                                         p_kf[:kw, c, q0:QA],
                                         start=first, stop=last)
                        if fold_b:
                            nc.tensor.matmul(o_b[:, :TAILW], v_t[:kw, c, :],
                                             p_kf[:kw, c, QA:S],
                                             start=first, stop=last)
                    ci += 2

                # seg-b (tail q columns) in TPB-chunk groups when not folded
                if TAILW and not fold_b:
                    ci = 0
                    while ci < NKC:
                        grp = list(range(ci, min(ci + TPB, NKC)))
                        st_t = att_st.tile([128, TPB, TAILW], dt.float32, tag="st")
                        for i, c in enumerate(grp):
                            ka, kb = KCH[c]
                            kw = kb - ka
                            nc.tensor.matmul(st_t[:kw, i, :],
                                             kr_t[:, ka:kb],
                                             qrot[:, h, QA:S],
                                             start=True, stop=True)
                        nfu = sum(1 for c in grp
                                  if KCH[c][1] - KCH[c][0] == 128)
                        if nfu:
                            nc.scalar.activation(
                                p_kf[:, ci:ci + nfu, QA:S],
                                st_t[:, :nfu, :],
                                AF.Exp, scale=inv_sqrt_d)
                        if nfu < len(grp):
                            nc.scalar.activation(
                                p_kf[:kw_tail, ci + nfu:ci + len(grp), QA:S],
                                st_t[:kw_tail, nfu:len(grp), :],
                                AF.Exp, scale=inv_sqrt_d)
                        for i, c in enumerate(grp):
                            ka, kb = KCH[c]
                            kw = kb - ka
                            nc.tensor.matmul(o_b[:, :TAILW], v_t[:kw, c, :],
                                             p_kf[:kw, c, QA:S],
                                             start=(kf == 0 and c == 0),
                                             stop=(kf == F - 1 and c == NKC - 1))
                        ci += TPB

                # softmax denominator: chunk-axis tree on DVE (level 1 in
                # bf16, the rest fp32), then one ones-matmul per q segment
                # reduces the 128 key partitions into the z bank.
                nh_ = NKC // 2
                t1 = att_t.tile([128, nh_, S], dt.bfloat16, tag="t1")
                nc.vector.tensor_add(t1[:, :, q0:S], p_kf[:, 0:nh_, q0:S],
                                     p_kf[:, nh_:2 * nh_, q0:S])
                n2 = nh_ // 2
                t2 = att_t.tile([128, n2, S], dt.float32, tag="t2")
                nc.vector.tensor_add(t2[:, :, q0:S], t1[:, 0:n2, q0:S],
                                     t1[:, n2:2 * n2, q0:S])
                for sl in range(1, n2):
                    nc.vector.tensor_add(t2[:, 0, q0:S], t2[:, 0, q0:S],
                                         t2[:, sl, q0:S])
                if nh_ % 2:  # odd level-1 slot
                    nc.vector.tensor_add(t2[:, 0, q0:S], t2[:, 0, q0:S],
                                         t1[:, nh_ - 1, q0:S])
                if NKC % 2:  # odd chunk (never paired at level 1)
                    nc.vector.tensor_add(t2[:, 0, q0:S], t2[:, 0, q0:S],
                                         p_kf[:, NKC - 1, q0:S])
                for zp, zlo, zhi in zrows:
                    ia, ib = max(q0, zlo), zhi
                    if ia >= ib:
                        continue
                    wkf = z_writers(zlo, zhi)
                    nc.tensor.matmul(
                        o_b[zp:zp + 1, TAILW + ia - zlo:TAILW + ib - zlo],
                        ones_f32[:, :], t2[:, 0, ia:ib],
                        start=(kf == wkf[0]), stop=(kf == wkf[-1]))

            # 1/Z and eviction for head h
            z_sb = att_m.tile([1, S], dt.float32, tag="zsb", name="zsb")
            nc.scalar.activation(z_sb[0:1, q0_min:ZS],
                                 o_b[0:1, TAILW:TAILW + ZS - q0_min], AF.Copy)
            nc.vector.tensor_copy(z_sb[0:1, ZS:S],
                                  o_b[32:33, TAILW:TAILW + S - ZS])
            nc.vector.reciprocal(z_sb[0:1, :], z_sb[0:1, :])
            z_dram = dram.tile([1, S], dt.float32, tag="zdram", bufs=2,
                               name="zdram")
            nc.sync.dma_start(out=z_dram[0:1, :], in_=z_sb[0:1, :])
            izb = att_m.tile([128, S], dt.float32, tag="izb", name="izb")
            nc.sync.dma_start(
                out=izb[:],
                in_=bass.AP(tensor=z_dram.tensor, offset=z_dram[0:1, :].offset,
                            ap=[[0, 128]] + z_dram[0:1, :].ap[1:]))
            nc.vector.tensor_mul(oT_sb[:, h, q0_min:QA],
                                 o_a[:, :], izb[:, q0_min:QA])
            if TAILW:
                nc.vector.tensor_mul(oT_sb[:, h, QA:S], o_b[:, :TAILW],
                                     izb[:, QA:S])

        actx.close()  # release attention PSUM banks before the O-projection

        # ---------------- O projection ----------------
        wopool = ctx.enter_context(tc.tile_pool(name="w_o", bufs=3))
        pspool = ctx.enter_context(
            tc.tile_pool(name="ps_o", bufs=len(TOKCH) + 1, space="PSUM"))
        evpool = ctx.enter_context(tc.tile_pool(name="ev_o", bufs=3))
        for sl in range(NSL):
            ps = {}
            for ti in range(len(TOKCH)):
                ps[ti] = pspool.tile([128, SLICE], dt.float32, tag="ops", name=f"ops{ti}")
            for m in range(NH):
                wt = wopool.tile([128, SLICE], dt.bfloat16, tag="wo")
                nc.sync.dma_start(
                    out=wt[:],
                    in_=w_d["woT"][m * 128:(m + 1) * 128,
                                   sl * SLICE:(sl + 1) * SLICE])
                for ti, (ta, tb) in enumerate(TOKCH):
                    nc.tensor.matmul(ps[ti][:tb - ta, :], oT_sb[:, m, ta:tb],
                                     wt[:], start=(m == 0), stop=(m == NH - 1))
            for ti, (ta, tb) in enumerate(TOKCH):
                tw = tb - ta
                ot = evpool.tile([128, SLICE], dt.float32, tag="oev")
                nc.vector.tensor_add(ot[:tw, :], ps[ti][:tw, :],
                                     bo_bc[:tw, sl * SLICE:(sl + 1) * SLICE])
                nc.sync.dma_start(
                    out=out_d[ta:tb, sl * SLICE:(sl + 1) * SLICE],
                    in_=ot[:tw, :])

    if cap_waits:
        _cap_sync_waits(nc, mybir)
    _BUILD_CACHE[key] = nc
    return nc


def _cap_sync_waits(nc, mybir, cap=1):
    """Walrus engine-instruction structs only have a limited number of sync
    wait slots.  Hoist excess waits onto InstNoOp carriers placed immediately
    before the instruction on the same engine stream."""
    exempt = (mybir.InstNoOp, mybir.InstEventSemaphore,
              mybir.InstAllEngineBarrier)
    for f in nc.m.functions:
        for bb in f.blocks:
            out = []
            changed = False
            for inst in bb.instructions:
                si = inst.sync_info
                if (si is None or len(si.on_wait) <= cap
                        or isinstance(inst, exempt)):
                    out.append(inst)
                    continue
                waits = list(si.on_wait)
                keep, excess = waits[:cap], waits[cap:]
                while excess:
                    batch, excess = excess[:cap], excess[cap:]
                    out.append(mybir.InstNoOp(
                        name=f"{inst.name}-w{len(out)}",
                        engine=inst.engine,
                        bass_nofuse=True,
                        sync_info=mybir.SyncInfo(on_wait=batch, on_update=[]),
                    ))
                inst.sync_info = mybir.SyncInfo(on_wait=keep,
                                                on_update=list(si.on_update))
                out.append(inst)
                changed = True
            if changed:
                bb.instructions = out


# ---------------------------------------------------------------------------
# host side
# ---------------------------------------------------------------------------
def _perm(NH):
    p = np.empty(NH * D, np.int64)
    for hh in range(NH):
        base = hh * D
        for j in range(D // 2):
            p[base + j] = base + 2 * j
            p[base + D // 2 + j] = base + 2 * j + 1
    return p


def _host_inputs(x, freqs, Wq, bq, Wk, bk, Wv, bv, Wo, bo, gq, gk,
                 f, h, w, num_heads, local_attn_size, sink_size, start_frame):
    NH = num_heads
    DIM = NH * D
    FRAME = h * w
    assert FRAME % NC == 0
    T = FRAME // NC
    S = f * T
    perm = _perm(NH)

    def bf(a):
        return np.ascontiguousarray(a, dtype=np.float32).astype(BF16)

    wqT = bf(Wq[perm].T)
    wkT = bf(Wk[perm].T)
    wvT = bf(Wv.T)
    woT = bf(Wo.T)
    def chunkmajor(a):
        return np.asarray(a, np.float32)[perm].reshape(NH, D).T
    bias_pack = np.ascontiguousarray(np.concatenate(
        [chunkmajor(bq), chunkmajor(gq), chunkmajor(bq) * chunkmajor(gq),
         chunkmajor(bk), chunkmajor(gk), chunkmajor(bk) * chunkmajor(gk)],
        axis=1), np.float32)
    bv_r = bf(bv.reshape(1, DIM))
    bo_r = np.ascontiguousarray(bo.reshape(1, DIM), np.float32)

    c = D // 2
    c1 = c // 3
    c0 = c - 2 * c1
    freqs = np.asarray(freqs, np.float32)

    in_maps = []
    tok_idx = []
    for core in range(NC):
        idx = np.concatenate(
            [fr * FRAME + T * core + np.arange(T) for fr in range(f)])
        tok_idx.append(idx)
        xT = bf(np.asarray(x[0], np.float32)[idx].T)
        fr = idx // FRAME
        rem = idx % FRAME
        hh_i = rem // w
        ww_i = rem % w
        ang = np.empty((c, S), np.float32)
        ang[:c0, :] = freqs[start_frame + fr][:, :c0].T
        ang[c0:c0 + c1, :] = freqs[hh_i][:, c0:c0 + c1].T
        ang[c0 + c1:, :] = freqs[ww_i][:, c0 + c1:c].T
        def wrap(a):
            a = np.asarray(a, np.float64)
            return (a - 2 * np.pi * np.round(a / (2 * np.pi))).astype(np.float32)
        # top half encodes -sin via the (ang + pi) phase shift
        angS = np.ascontiguousarray(
            np.concatenate([wrap(ang + np.pi), wrap(ang)], 0), np.float32)
        angC = np.ascontiguousarray(
            np.concatenate([wrap(ang + np.pi / 2), wrap(ang + np.pi / 2)], 0),
            np.float32)
        in_maps.append({
            "xT": xT, "wqT": wqT, "wkT": wkT, "wvT": wvT, "woT": woT,
            "bias_pack": bias_pack,
            "bv_r": bv_r, "bo_r": bo_r, "angS": angS, "angC": angC,
        })
    return in_maps, tok_idx, T, S


def _allowed(f, local_attn_size, sink_size):
    return [
        [kf for kf in range(f)
         if kf <= qf and (qf - kf < local_attn_size or kf < sink_size)]
        for qf in range(f)
    ]


def kernel(x, freqs, Wq, bq, Wk, bk, Wv, bv, Wo, bo, gq, gk,
           f, h, w, num_heads, local_attn_size, sink_size, start_frame,
           _trace=False):
    from concourse.bass_utils import run_bass_kernel_spmd

    f = int(f); h = int(h); w = int(w)
    num_heads = int(num_heads)
    local_attn_size = int(local_attn_size)
    sink_size = int(sink_size)
    start_frame = int(start_frame)

    x = np.asarray(x)
    B, L, DIM = x.shape
    assert B == 1 and DIM == num_heads * D

    allowed = _allowed(f, local_attn_size, sink_size)
    in_maps, tok_idx, T, S = _host_inputs(
        x, freqs, Wq, bq, Wk, bk, Wv, bv, Wo, bo, gq, gk,
        f, h, w, num_heads, local_attn_size, sink_size, start_frame)
    nc = build_program(num_heads, f, T, allowed)
    res = run_bass_kernel_spmd(nc, in_maps, core_ids=list(range(NC)),
                               trace=_trace)
    out = np.empty((1, L, DIM), np.float32)
    for core in range(NC):
        out[0, tok_idx[core]] = res.results[core]["out"]
    if _trace:
        kernel._last_results = res
    return out
